# revision 1
# baseline (speedup 1.0000x reference)
"""Trainium2 Bass kernel for nn_GAttn_67147518705771.

Computes: score = w0*RBF(gf0, s0) + w1*RBF(gf1, s1)  (N x N)
          attn  = score / (rowsum(score) + 0.01)
          out   = attn @ V + V

Sharding: row-parallel over 8 NeuronCores — core c computes output rows
[c*1024, (c+1)*1024); the key/value side (all N=8192 nodes) is replicated.

Per-core algorithm (all on-chip, no N^2 HBM traffic):
  The exp argument E_m[j,i] = -d2_m[j,i]/(2*sigma_m^2) + ln(w_m) is produced
  directly by ONE bf16 matmul per modality with K=24 augmented feature rows
  (3-way bf16 hi/mid/lo splits of the cross/sq terms give ~fp32 accuracy while
  streaming at bf16 speed, 1 col/cycle). ScalarE exponentiates PSUM->SBUF
  (fp16) in 1536-element ops spanning 3 j-blocks x 2 modalities, DVE adds the
  two modalities, and PE accumulates S @ [V | 1] into persistent PSUM, which
  yields the row sums (ones column) for free. The division by (rowsum + eps)
  commutes with @V, so it is applied to the [128,129] accumulators only, then
  the residual is added and rows stored.

PSUM (8 banks): E tiles [128,1536] (3 banks) double-buffered = 6, U
accumulator [128,258] (1 bank) double-buffered across i-chunks = 2.
start=True clears has_written for the WHOLE bank, so only the first matmul
per accumulation bank sets it.
"""

import numpy as np
import ml_dtypes

import concourse.bass as bass
import concourse.tile as tile
import concourse.mybir as mybir

BF16 = ml_dtypes.bfloat16
EPS = 0.01
N = 8192          # total nodes (j / key dim)
DG = 3            # geometric feature dim
DV = 128          # value dim
NCORES = 8
NI = N // NCORES  # query rows per core (1024)
KF = 24           # feature rows per modality
CHUNK = 256       # i-chunk per pass (4 passes per core)
GRP = 3           # j-blocks per E tile / ACT op
# fp16 Schraudolph exp for the DVE-offloaded share: bits = uint16 convert of
# E*A + B (round-to-nearest, negatives saturate to +0 = underflow-exact),
# bitcast to fp16. Max rel err ~3% where S>1e-3; end-to-end ~9e-4.
SCH_A = float(np.float32(1024.0 / np.log(2.0)))
SCH_B = float(np.float32(15360.0 - 45.0))
DVE_EXP_EVERY = 0  # disabled: in-order DVE/GPSIMD queues serialize the offload


def _split_sync_waits(nc, maxw=1):
    """The walrus build in this environment rejects instructions carrying
    more than one sync wait ("Too many sync wait commands"). Hoist excess
    waits onto single-wait InstNoOp carriers inserted just before the owning
    instruction (same engine => same sequencer stream, so ordering-equivalent).

    The kernel-tail drain (an SP InstDrain carrying the whole global clock,
    followed by the all-engine barrier) gets its waits distributed round-robin
    across ALL engine sequencers instead, so they are satisfied in parallel;
    the subsequent barrier keeps this ordering-equivalent.
    Returns (n_insts_split, n_carriers)."""
    n_split = n_carriers = 0
    eng_rr = [
        mybir.EngineType.SP,
        mybir.EngineType.Activation,
        mybir.EngineType.DVE,
        mybir.EngineType.PE,
        mybir.EngineType.Pool,
    ]
    for f in nc.m.functions:
        for bb in f.blocks:
            insts = list(bb.instructions)
            out, changed = [], False
            for inst in insts:
                si = inst.sync_info
                waits = list(si.on_wait) if si and si.on_wait else []
                if len(waits) > maxw:
                    n_split += 1
                    changed = True
                    is_tail_drain = (
                        isinstance(inst, mybir.InstDrain)
                        and inst.engine == mybir.EngineType.SP
                        and len(waits) > 2
                    )
                    for k, w in enumerate(waits[:-maxw]):
                        nop = mybir.InstNoOp(name=f"waitnop-{n_carriers}", ins=[], outs=[])
                        n_carriers += 1
                        nop.engine = eng_rr[k % len(eng_rr)] if is_tail_drain else inst.engine
                        nop.sync_info = mybir.SyncInfo(on_wait=[w], on_update=[])
                        out.append(nop)
                    inst.sync_info = mybir.SyncInfo(
                        on_wait=waits[-maxw:], on_update=list(si.on_update or [])
                    )
                out.append(inst)
            if changed:
                bb.instructions = out
    return n_split, n_carriers


def build_nc(n_j=N, n_i=NI):
    """Build the per-core Bass program (SPMD: same program, per-core data)."""
    f32 = mybir.dt.float32
    f16 = mybir.dt.float16
    bf16 = mybir.dt.bfloat16
    njb = n_j // 128
    nchunks = n_i // CHUNK
    # resident input tiles are split into <=8 even pieces for fine-grained
    # DMA->compute overlap at startup.
    step = (njb + 7) // 8
    piece_start = list(range(0, njb, step))
    piece_of = [min(j // step, len(piece_start) - 1) for j in range(njb)]

    nc = bass.Bass("TRN2", target_bir_lowering=False, debug=False)
    L0 = nc.dram_tensor("L0", [KF, n_j], bf16, kind="ExternalInput").ap()
    L1 = nc.dram_tensor("L1", [KF, n_j], bf16, kind="ExternalInput").ap()
    R0 = nc.dram_tensor("R0", [KF, n_i], bf16, kind="ExternalInput").ap()
    R1 = nc.dram_tensor("R1", [KF, n_i], bf16, kind="ExternalInput").ap()
    # V_aug pre-rearranged on host: [128, njb*129] fp16, block jb holds rows
    # jb*128..jb*128+127 of [V | 1].
    VA = nc.dram_tensor("VA", [128, njb * 129], f16, kind="ExternalInput").ap()
    # V residual rows for this core, 128-row-block-major: [128, n_i] f32.
    VR = nc.dram_tensor("VR", [128, n_i], f32, kind="ExternalInput").ap()
    OUT = nc.dram_tensor("out", [n_i, DV], f32, kind="ExternalOutput").ap()

    # First group takes the remainder so (a) the first exp has minimal
    # dependencies and starts early, (b) the kernel tail ends on full groups.
    first = (njb - 1) % GRP + 1
    groups = [list(range(0, first))] + [
        list(range(g, g + GRP)) for g in range(first, njb, GRP)
    ]

    with tile.TileContext(nc) as tc:
        with (
            tc.tile_pool(name="resident", bufs=1) as rpool,
            tc.tile_pool(name="eapool", bufs=2, space="PSUM") as eapool,
            tc.tile_pool(name="ebpool", bufs=2, space="PSUM") as ebpool,
            tc.tile_pool(name="upool", bufs=2, space="PSUM") as upool,
            tc.tile_pool(name="spool", bufs=6) as spool,
            tc.tile_pool(name="sumpool", bufs=5) as sumpool,
            tc.tile_pool(name="opool", bufs=4) as opool,
            tc.tile_pool(name="scalars", bufs=4) as scpool,
        ):
            # Spread the first group's operands across independent DMA-issue
            # rails (each rail serializes at ~0.6-1.3us per dma_start). The
            # first E matmul's LDWEIGHTS needs l0 piece 0 -> it goes FIRST on
            # the sync rail, r0 second; l1 piece 0 on the scalar/ACT rail;
            # r1 + va piece 0 on gpsimd. Later pieces interleave on sync in
            # compute order.
            l_tiles = [[], []]
            va_tiles = []

            def piece_tiles(idx, st):
                en = piece_start[idx + 1] if idx + 1 < len(piece_start) else njb
                w = en - st
                lt0 = rpool.tile([KF, w * 128], bf16, name=f"l0_{st}")
                lt1 = rpool.tile([KF, w * 128], bf16, name=f"l1_{st}")
                vt = rpool.tile([128, w * 129], f16, name=f"va_{st}")
                return lt0, lt1, vt, en

            lt0, lt1, vt, en0 = piece_tiles(0, 0)
            nc.sync.dma_start(lt0[:], L0[:, 0:en0 * 128])
            nc.scalar.dma_start(lt1[:], L1[:, 0:en0 * 128])
            l_tiles[0].append(lt0)
            l_tiles[1].append(lt1)
            r0_sb = rpool.tile([KF, n_i], bf16)
            nc.sync.dma_start(r0_sb[:], R0[:])
            r1_sb = rpool.tile([KF, n_i], bf16)
            nc.gpsimd.dma_start(r1_sb[:], R1[:])
            nc.gpsimd.dma_start(vt[:], VA[:, 0:en0 * 129])
            va_tiles.append(vt)

            for idx in range(1, len(piece_start)):
                st = piece_start[idx]
                lt0, lt1, vt, en = piece_tiles(idx, st)
                nc.sync.dma_start(lt0[:], L0[:, st * 128:en * 128])
                nc.sync.dma_start(lt1[:], L1[:, st * 128:en * 128])
                nc.sync.dma_start(vt[:], VA[:, st * 129:en * 129])
                l_tiles[0].append(lt0)
                l_tiles[1].append(lt1)
                va_tiles.append(vt)

            vr_sb = rpool.tile([128, n_i], f32)
            nc.sync.dma_start(vr_sb[:], VR[:])

            # Dummy exp (after the ACT-rail DMA issue) pre-loads the ACT
            # exp-table while the input DMAs stream in.
            dummy = scpool.tile([128, 1], f32, tag="dummy")
            nc.vector.memset(dummy[:], 0.0)
            dummy2 = scpool.tile([128, 1], f32, tag="dummy2")
            nc.scalar.activation(dummy2[:], dummy[:], mybir.ActivationFunctionType.Exp)

            # A few dummy matmuls during the input-DMA wait start the PE HAM
            # warm-up early so the first real E matmuls run at a higher clock.
            dmm = scpool.tile([1, 256], bf16, tag="dmm")
            nc.vector.memset(dmm[:], 0.0)
            e_warm = eapool.tile([128, 1024], f32, tag="ea", name="e_warm")
            for k in range(4):
                nc.tensor.matmul(
                    e_warm[:, 0:256], lhsT=dmm[:, 0:128], rhs=dmm[:, 0:256],
                    start=True, stop=True,
                )

            def lsl(m, jb):  # lhsT feature slice [KF, 128] for modality m
                idx = piece_of[jb]
                o = (jb - piece_start[idx]) * 128
                return l_tiles[m][idx][:, o:o + 128]

            def vasl(jb):  # V_aug block [128, 129]
                idx = piece_of[jb]
                o = (jb - piece_start[idx]) * 129
                return va_tiles[idx][:, o:o + 129]

            # Chunks are processed in interleaved PAIRS: while chunk c0's exp
            # runs on ScalarE, the PE computes chunk c1's E matmuls, so the
            # PE program order never stalls on the last exp of a chunk except
            # at pair boundaries. Accumulation matmuls are emitted one work
            # item late (deferred) so the PE's in-order stream reaches the
            # next group's E matmuls before stalling on the current exp.
            assert nchunks % 2 == 0
            npairs = nchunks // 2
            for cpair in range(npairs):
                chunks = (2 * cpair, 2 * cpair + 1)
                # Last pair: rotate the small remainder group to the END so
                # the final exp->accumulate->epilogue chain is minimal.
                pgroups = groups if cpair < npairs - 1 else groups[1:] + groups[:1]
                order = {0: pgroups, 1: pgroups}
                # Per-chunk U accumulator: 2 subblocks x 129 cols in ONE bank.
                u_t = [upool.tile([128, 2 * 129], f32, tag="u", name=f"u_{c}")
                       for c in chunks]
                deferred = []

                def emit_accums(item):
                    u, ss, sub, jbs, first_grp, last_grp = item
                    for t, jb in enumerate(jbs):
                        for isub in range(2):
                            # start=True clears has_written for the WHOLE
                            # bank: only the first matmul touching the bank
                            # in this chunk sets it; later first-writes land
                            # on cleared bits (= overwrite), then accumulate.
                            if t < 2:
                                lhs = [ss[:, t * 256 + isub * 128:
                                          t * 256 + (isub + 1) * 128]]
                            else:
                                lhs = [sub[:, m * 256 + isub * 128:
                                           m * 256 + (isub + 1) * 128]
                                       for m in range(2)]
                            for li, lt in enumerate(lhs):
                                nc.tensor.matmul(
                                    u[:, isub * 129:(isub + 1) * 129],
                                    lhsT=lt,
                                    rhs=vasl(jb),
                                    start=(first_grp and t == 0 and isub == 0
                                           and li == 0),
                                    stop=(last_grp and t == len(jbs) - 1),
                                    skip_group_check=True,
                                )

                def emit_epilogue(ci, chunk):
                    for isub in range(2):
                        g = chunk * 2 + isub
                        ut = u_t[ci][:, isub * 129:(isub + 1) * 129]
                        rt = scpool.tile([128, 1], f32, tag="rt", name=f"rt_{g}")
                        nc.vector.tensor_scalar_add(rt[:], ut[:, 128:129], EPS)
                        ri = scpool.tile([128, 1], f32, tag="ri", name=f"ri_{g}")
                        nc.vector.reciprocal(ri[:], rt[:])
                        ot = opool.tile([128, DV], f32, tag="ot", name=f"ot_{g}")
                        nc.vector.tensor_scalar_mul(ot[:], ut[:, 0:DV], ri[:])
                        nc.vector.tensor_add(ot[:], ot[:], vr_sb[:, g * 128:(g + 1) * 128])
                        out_eng = nc.sync if isub == 0 else nc.gpsimd
                        out_eng.dma_start(OUT[g * 128:(g + 1) * 128, :], ot[:])

                items = [(order[ci][k], ci)
                         for k in range(len(groups)) for ci in (0, 1)]
                seen = {0: 0, 1: 0}
                wi = [0]

                def work(item):
                    jbs, ci = item
                    chunk = chunks[ci]
                    c0 = chunk * CHUNK
                    wi[0] += 1
                    wa = min(len(jbs), 2) * 512
                    ea = eapool.tile([128, 1024], f32, tag="ea",
                                     name=f"ea_{chunk}_{jbs[0]}")
                    eb = (ebpool.tile([128, 512], f32, tag="eb",
                                      name=f"eb_{chunk}_{jbs[0]}")
                          if len(jbs) > 2 else None)
                    for t, jb in enumerate(jbs):
                        for m, rsb in ((0, r0_sb), (1, r1_sb)):
                            dst = (ea[:, t * 512 + m * 256:t * 512 + (m + 1) * 256]
                                   if t < 2 else eb[:, m * 256:(m + 1) * 256])
                            nc.tensor.matmul(
                                dst, lhsT=lsl(m, jb),
                                rhs=rsb[:, c0:c0 + CHUNK],
                                start=True, stop=True,
                            )
                    sa = spool.tile([128, 1024], f16, tag="s",
                                    name=f"s_{chunk}_{jbs[0]}")
                    nc.scalar.activation(
                        sa[:, 0:wa], ea[:, 0:wa], mybir.ActivationFunctionType.Exp
                    )
                    if eb is not None:
                        su = spool.tile([128, 512], mybir.dt.uint16, tag="su",
                                        name=f"su_{chunk}_{jbs[0]}")
                        nc.vector.tensor_scalar(
                            su[:], eb[:], SCH_A, SCH_B,
                            mybir.AluOpType.mult, mybir.AluOpType.add,
                        )
                        sub = su[:].bitcast(f16)
                    else:
                        sub = None
                    ss = sumpool.tile([128, 512], f16, tag="ss",
                                      name=f"ss_{chunk}_{jbs[0]}")
                    for t in range(min(len(jbs), 2)):
                        nc.vector.tensor_add(
                            ss[:, t * 256:(t + 1) * 256],
                            sa[:, t * 512:t * 512 + 256],
                            sa[:, t * 512 + 256:(t + 1) * 512],
                        )
                    seen[ci] += 1
                    return (u_t[ci], ss, sub, jbs, seen[ci] == 1,
                            seen[ci] == len(groups), ci, chunk)

                def retire(item):
                    emit_accums(item[:6])

                for item in items:
                    deferred.append(work(item))
                    if len(deferred) > 2:
                        retire(deferred.pop(0))
                while deferred:
                    retire(deferred.pop(0))
                for ci, chunk in enumerate(chunks):
                    emit_epilogue(ci, chunk)

    _split_sync_waits(nc)
    return nc


def _split3(v):
    v1 = v.astype(BF16).astype(np.float32)
    v2 = (v - v1).astype(BF16).astype(np.float32)
    v3 = (v - v1 - v2).astype(BF16).astype(np.float32)
    return v1, v2, v3


def _build_features(gf, sigma, w):
    """L [KF, N] (j-side) and R [KF, N] (i-side) bf16 feature rows such that
    (L.T @ R)[j, i] = -d2[j,i]/(2 sigma^2) + ln(w) to ~1e-5."""
    gf = np.asarray(gf, dtype=np.float32)
    n = gf.shape[0]
    g = np.float32(1.0 / (2.0 * sigma * sigma))
    sq = (gf * gf).sum(axis=1)
    a = 2.0 * g * gf            # j-side cross
    b = gf                      # i-side cross
    dterm = -g * sq             # j-side
    c = -g * sq + np.float32(np.log(w))  # i-side

    a1, a2, a3 = _split3(a)
    b1, b2, b3 = _split3(b)
    d1, d2_, d3 = _split3(dterm)
    c1, c2, c3 = _split3(c)
    ones = np.ones(n, np.float32)

    Lrows, Rrows = [], []
    for ap, bp in [(a1, b1), (a1, b2), (a2, b1), (a2, b2), (a3, b1), (a1, b3)]:
        for d in range(DG):
            Lrows.append(ap[:, d])
            Rrows.append(bp[:, d])
    for dd in (d1, d2_, d3):
        Lrows.append(dd)
        Rrows.append(ones)
    for cc in (c1, c2, c3):
        Lrows.append(ones)
        Rrows.append(cc)
    L = np.stack(Lrows).astype(BF16)
    R = np.stack(Rrows).astype(BF16)
    return L, R


def _prepare_inputs(gf0, gf1, node_v_feats, weights, sigmas, n_cores=NCORES):
    """Host-side preprocessing -> per-core in_maps."""
    weights = np.asarray(weights, np.float32)
    sigmas = np.asarray(sigmas, np.float32)
    V = np.asarray(node_v_feats, np.float32)
    n = V.shape[0]
    ni = n // n_cores
    njb = n // 128

    L0, R0full = _build_features(gf0, float(sigmas[0]), float(weights[0]))
    L1, R1full = _build_features(gf1, float(sigmas[1]), float(weights[1]))

    vaug = np.concatenate([V, np.ones((n, 1), np.float32)], axis=1)  # [n, 129]
    va = np.ascontiguousarray(
        vaug.reshape(njb, 128, 129).transpose(1, 0, 2).reshape(128, njb * 129)
    ).astype(np.float16)

    in_maps = []
    for c in range(n_cores):
        rows = slice(c * ni, (c + 1) * ni)
        vr = np.ascontiguousarray(
            V[rows].reshape(ni // 128, 128, DV).transpose(1, 0, 2).reshape(128, ni)
        )
        in_maps.append({
            "L0": np.ascontiguousarray(L0),
            "L1": np.ascontiguousarray(L1),
            "R0": np.ascontiguousarray(R0full[:, rows]),
            "R1": np.ascontiguousarray(R1full[:, rows]),
            "VA": va,
            "VR": vr,
        })
    return in_maps


_NC_CACHE = {}


def _get_nc(n_j=N, n_i=NI):
    key = (n_j, n_i)
    if key not in _NC_CACHE:
        _NC_CACHE[key] = build_nc(n_j, n_i)
    return _NC_CACHE[key]


_EXEC_CACHE = {}


def _get_executor(nc, n_cores):
    """Cached jitted shard_map executor (avoids re-tracing per call)."""
    key = (id(nc), n_cores)
    if key in _EXEC_CACHE:
        return _EXEC_CACHE[key]
    import jax
    from jax.experimental.shard_map import shard_map
    from jax.sharding import Mesh, PartitionSpec
    from concourse.bass2jax import (
        install_neuronx_cc_hook,
        _bass_exec_p,
        partition_id_tensor,
    )

    install_neuronx_cc_hook()

    partition_name = nc.partition_id_tensor.name if nc.partition_id_tensor else None
    in_names, out_names, out_avals = [], [], []
    for alloc in nc.m.functions[0].allocations:
        if not isinstance(alloc, mybir.MemoryLocationSet):
            continue
        name = alloc.memorylocations[0].name
        if alloc.kind == "ExternalInput":
            if name != partition_name:
                in_names.append(name)
        elif alloc.kind == "ExternalOutput":
            out_names.append(name)
            out_avals.append(
                jax.core.ShapedArray(tuple(alloc.tensor_shape), mybir.dt.np(alloc.dtype))
            )
    n_params = len(in_names)
    all_names = list(in_names) + list(out_names)
    if partition_name is not None:
        all_names.append(partition_name)

    def _body(*args):
        operands = list(args)
        if partition_name is not None:
            operands.append(partition_id_tensor())
        outs = _bass_exec_p.bind(
            *operands,
            out_avals=tuple(out_avals),
            in_names=tuple(all_names),
            out_names=tuple(out_names),
            lowering_input_output_aliases=(),
            sim_require_finite=True,
            sim_require_nnan=True,
            nc=nc,
        )
        return tuple(outs)

    devices = jax.devices()[:n_cores]
    mesh = Mesh(np.asarray(devices), ("core",))
    n_outs = len(out_names)
    replicated = frozenset(["L0", "L1", "VA"])  # identical across cores
    in_specs = tuple(
        PartitionSpec() if name in replicated else PartitionSpec("core")
        for name in in_names
    ) + (PartitionSpec("core"),) * n_outs
    sharded = jax.jit(
        shard_map(
            _body,
            mesh=mesh,
            in_specs=in_specs,
            out_specs=(PartitionSpec("core"),) * n_outs,
            check_rep=False,
        ),
        donate_argnums=tuple(range(n_params, n_params + n_outs)),
        keep_unused=True,
    )
    entry = (sharded, in_names, out_names, out_avals, replicated)
    _EXEC_CACHE[key] = entry
    return entry


def _run(nc, in_maps, n_cores):
    sharded, in_names, out_names, out_avals, replicated = _get_executor(nc, n_cores)
    concat_in = [
        in_maps[0][name] if name in replicated
        else np.concatenate([in_maps[c][name] for c in range(n_cores)], axis=0)
        for name in in_names
    ]
    concat_zeros = [
        np.zeros((n_cores * a.shape[0], *a.shape[1:]), a.dtype) for a in out_avals
    ]
    out_arrs = sharded(*concat_in, *concat_zeros)
    return [
        {
            name: np.asarray(out_arrs[i]).reshape(n_cores, *out_avals[i].shape)[c]
            for i, name in enumerate(out_names)
        }
        for c in range(n_cores)
    ]


def kernel(gf0, gf1, node_v_feats, weights, sigmas):
    import jax

    in_maps = _prepare_inputs(gf0, gf1, node_v_feats, weights, sigmas)
    nc = _get_nc()
    last_exc = None
    for attempt in range(3):
        try:
            results = _run(nc, in_maps, NCORES)
            # Surface any async device failure here (rare transient
            # NRT_EXEC_UNIT_UNRECOVERABLE) instead of at interpreter exit.
            jax.effects_barrier()
            out = np.concatenate([results[c]["out"] for c in range(NCORES)], axis=0)
            return np.ascontiguousarray(out.astype(np.float32))
        except Exception as e:  # retry once with a fresh backend/executor
            last_exc = e
            _EXEC_CACHE.clear()
            try:
                jax.clear_caches()
            except Exception:
                pass
            try:
                jax._src.xla_bridge.backends.cache_clear()  # type: ignore[attr-defined]
            except Exception:
                pass
            import time as _time
            _time.sleep(5 * (attempt + 1))
    raise last_exc



# revision 2
# speedup vs baseline: 1.0134x; 1.0134x over previous
"""Trainium2 Bass kernel v2 for nn_GAttn_67147518705771.

Computes: score = w0*RBF(gf0, s0) + w1*RBF(gf1, s1)  (N x N)
          attn  = score / (rowsum(score) + 0.01)
          out   = attn @ V + V

Row-parallel over 8 cores; core c owns output rows [c*1024, (c+1)*1024).

v2 strategy (vs the f16 baseline):
  * All matmuls are fp8e4 + DoubleRow (0.5 cyc/col, 256-row contraction):
    - E matmul per modality: 24 e4m3 feature rows as [12, 2, *] pairs
      produce E[j,i]*1 with ln(w) + 5*ln2 folded in (S scaled x32 so the
      e4m3 subnormal band sits harmlessly low; epilogue uses EPS*32).
    - Accumulation pairs the TWO MODALITY S tiles in one DoubleRow matmul
      against a byte-duplicated VA block [128, 2, 129] ([V|1] e4m3), so
      S0@VA + S1@VA costs 64.5 PE cycles per (jb, i128) block.
  * exp runs on ACT (native Exp -> fp8 out) and DVE (Schraudolph affine ->
    u8, bitcast e4m3). GPSIMD has no PSUM port so it only issues DMAs.
  * PSUM: 3 x [128,4,256] E tiles (2 banks each) + 2 x U accumulator banks.
"""

import numpy as np
import ml_dtypes

import concourse.bass as bass
import concourse.tile as tile
import concourse.mybir as mybir

E4NP = ml_dtypes.float8_e4m3
EPS = 0.01
N = 8192
DG = 3
DV = 128
NCORES = 8
NI = N // NCORES
KI = 12            # feature pair-rows per modality (24 rows via DoubleRow)
CHUNK = 256
NJB = N // 128
SCALE_OCT = 5      # S scaled by 2^5; epilogue divides by (rowsum + EPS*32)
EPS_S = EPS * (1 << SCALE_OCT)
SCH_A = float(np.float32(8.0 * 1.4426950408889634))
SCH_B = float(np.float32(55.537))
DEFER = 3          # groups between exp emission and its accum matmuls

# per-group exp cost estimates (ns) for greedy ACT/DVE balancing
COST = {
    ("act", 1): 612.0, ("act", 2): 1038.0,
    ("dve", 1): 658.0, ("dve", 2): 1192.0,
}


def _split_sync_waits(nc, maxw=1):
    """Walrus rejects instructions with >1 sync waits. Hoist extras onto
    single-wait InstNoOp carriers (same engine). The kernel-tail SP drain's
    waits are spread round-robin across all engines (barrier follows)."""
    n_split = n_carriers = 0
    eng_rr = [
        mybir.EngineType.SP,
        mybir.EngineType.Activation,
        mybir.EngineType.DVE,
        mybir.EngineType.PE,
        mybir.EngineType.Pool,
    ]
    for f in nc.m.functions:
        for bb in f.blocks:
            insts = list(bb.instructions)
            out, changed = [], False
            for inst in insts:
                si = inst.sync_info
                waits = list(si.on_wait) if si and si.on_wait else []
                if len(waits) > maxw:
                    n_split += 1
                    changed = True
                    is_tail_drain = (
                        isinstance(inst, mybir.InstDrain)
                        and inst.engine == mybir.EngineType.SP
                        and len(waits) > 2
                    )
                    for k, w in enumerate(waits[:-maxw]):
                        nop = mybir.InstNoOp(name=f"waitnop-{n_carriers}", ins=[], outs=[])
                        n_carriers += 1
                        nop.engine = eng_rr[k % len(eng_rr)] if is_tail_drain else inst.engine
                        nop.sync_info = mybir.SyncInfo(on_wait=[w], on_update=[])
                        out.append(nop)
                    inst.sync_info = mybir.SyncInfo(
                        on_wait=waits[-maxw:], on_update=list(si.on_update or [])
                    )
                out.append(inst)
            if changed:
                bb.instructions = out
    return n_split, n_carriers


def build_nc(n_j=N, n_i=NI):
    f32 = mybir.dt.float32
    fp8 = mybir.dt.float8e4
    u8 = mybir.dt.uint8
    njb = n_j // 128
    nchunks = n_i // CHUNK

    nc = bass.Bass("TRN2", target_bir_lowering=False, debug=False)
    L0 = nc.dram_tensor("L0", [KI, 2, n_j], fp8, kind="ExternalInput").ap()
    L1 = nc.dram_tensor("L1", [KI, 2, n_j], fp8, kind="ExternalInput").ap()
    R0 = nc.dram_tensor("R0", [KI, 2, n_i], fp8, kind="ExternalInput").ap()
    R1 = nc.dram_tensor("R1", [KI, 2, n_i], fp8, kind="ExternalInput").ap()
    # VA[p, 2*jb+m, v] = [V|1] row (jb*128+p), col v — duplicated per pair m.
    VA = nc.dram_tensor("VA", [128, 2 * njb, 129], fp8, kind="ExternalInput").ap()
    VR = nc.dram_tensor("VR", [128, n_i], f32, kind="ExternalInput").ap()
    OUT = nc.dram_tensor("out", [n_i, DV], f32, kind="ExternalOutput").ap()

    # per-chunk groups: first chunk leads with a single j-block so the first
    # exp starts early; last chunk ends with singles for a short tail drain.
    g_first = [[0]] + [[j, j + 1] for j in range(1, njb - 1, 2)] + [[njb - 1]]
    g_mid = [[j, j + 1] for j in range(0, njb, 2)]
    g_last = [[j, j + 1] for j in range(0, njb - 2, 2)] + [[njb - 2], [njb - 1]]

    with tile.TileContext(nc) as tc:
        with (
            tc.tile_pool(name="resident", bufs=1) as rpool,
            tc.tile_pool(name="eapool", bufs=3, space="PSUM") as eapool,
            tc.tile_pool(name="upool", bufs=2, space="PSUM") as upool,
            tc.tile_pool(name="spool", bufs=6) as spool,
            tc.tile_pool(name="opool", bufs=4) as opool,
            tc.tile_pool(name="scalars", bufs=4) as scpool,
        ):
            # --- resident inputs, spread across DMA-issue rails ---
            l0_t = rpool.tile([KI, 2, n_j], fp8)
            l1_t = rpool.tile([KI, 2, n_j], fp8)
            r0_t = rpool.tile([KI, 2, n_i], fp8)
            r1_t = rpool.tile([KI, 2, n_i], fp8)
            va_t = rpool.tile([128, 2 * njb, 129], fp8)
            vr_t = rpool.tile([128, n_i], f32)

            PIECE = 8  # j-blocks per L piece
            nc.sync.dma_start(r0_t[:], R0)
            nc.sync.dma_start(l0_t[:, :, 0:PIECE * 128], L0[:, :, 0:PIECE * 128])
            nc.scalar.dma_start(r1_t[:], R1)
            nc.scalar.dma_start(l1_t[:, :, 0:PIECE * 128], L1[:, :, 0:PIECE * 128])
            nc.gpsimd.dma_start(va_t[:, 0:2 * PIECE, :], VA[:, 0:2 * PIECE, :])
            for p in range(PIECE, njb, PIECE):
                nc.sync.dma_start(
                    l0_t[:, :, p * 128:(p + PIECE) * 128],
                    L0[:, :, p * 128:(p + PIECE) * 128],
                )
                nc.sync.dma_start(
                    l1_t[:, :, p * 128:(p + PIECE) * 128],
                    L1[:, :, p * 128:(p + PIECE) * 128],
                )
                nc.gpsimd.dma_start(
                    va_t[:, 2 * p:2 * (p + PIECE), :], VA[:, 2 * p:2 * (p + PIECE), :]
                )
            nc.sync.dma_start(vr_t[:], VR)

            # ACT exp-table preload + PE p-state warm-up during input DMA.
            dummy = scpool.tile([128, 1], f32, tag="dummy")
            nc.vector.memset(dummy[:], 0.0)
            dummy2 = scpool.tile([128, 1], f32, tag="dummy2")
            nc.scalar.activation(dummy2[:], dummy[:], mybir.ActivationFunctionType.Exp)
            dmm = scpool.tile([1, 256], mybir.dt.bfloat16, tag="dmm")
            nc.vector.memset(dmm[:], 0.0)
            e_warm = eapool.tile([128, 4, 256], f32, tag="ea", name="e_warm")
            for _ in range(14):
                nc.tensor.matmul(
                    e_warm[:, 0, :], lhsT=dmm[:, 0:128], rhs=dmm[:, 0:256],
                    start=True, stop=True,
                )

            # --- global stream of (chunk, group) items with greedy ACT/DVE
            # balance; accum matmuls deferred DEFER groups so the PE never
            # stalls on an in-flight exp, including across chunk boundaries.
            chunk_groups = [g_first] + [g_mid] * (nchunks - 2) + [g_last]
            items = [(c, gi) for c in range(nchunks)
                     for gi in range(len(chunk_groups[c]))]
            t_eng = {"act": 0.0, "dve": 0.0}
            u_tiles = {}
            deferred = []

            def emit_accums(item):
                c, s_t, jbs, first, last = item
                u_t = u_tiles[c]
                for t, jb in enumerate(jbs):
                    for isub in range(2):
                        # DR psum writes need >=1KB-aligned offsets: slots at
                        # f32 cols 0 and 256 of a full bank.
                        nc.tensor.matmul(
                            u_t[:, isub * 256:isub * 256 + 129],
                            lhsT=s_t[:, 2 * t:2 * t + 2,
                                     isub * 128:(isub + 1) * 128].bitcast(
                                         mybir.dt.float8e4),
                            rhs=va_t[:, 2 * jb:2 * jb + 2, :],
                            start=(first and t == 0 and isub == 0),
                            stop=(last and t == len(jbs) - 1 and isub == 1),
                            skip_group_check=True,
                            perf_mode=mybir.MatmulPerfMode.DoubleRow,
                        )
                if last:
                    emit_epilogue(c)

            def emit_epilogue(c):
                # out rows = U/(rowsum+EPS_S) + V residual; residual add and
                # store run on Pool/SP rails, PSUM-side ops on DVE.
                u_t = u_tiles.pop(c)
                for isub in range(2):
                    g = c * 2 + isub
                    ub = u_t[:, isub * 256:isub * 256 + 129]
                    rt = scpool.tile([128, 1], f32, tag="rt", name=f"rt_{g}")
                    nc.vector.tensor_scalar_add(rt[:], ub[:, 128:129], EPS_S)
                    ri = scpool.tile([128, 1], f32, tag="ri", name=f"ri_{g}")
                    nc.vector.reciprocal(ri[:], rt[:])
                    ot = opool.tile([128, DV], f32, tag="ot", name=f"ot_{g}")
                    nc.vector.tensor_scalar_mul(ot[:], ub[:, 0:DV], ri[:])
                    nc.gpsimd.tensor_add(ot[:], ot[:],
                                         vr_t[:, g * 128:(g + 1) * 128])
                    out_eng = nc.sync if isub == 0 else nc.gpsimd
                    out_eng.dma_start(OUT[g * 128:(g + 1) * 128, :], ot[:])

            def work(c, gi):
                jbs = chunk_groups[c][gi]
                nt = len(jbs)
                c0 = c * CHUNK
                if gi == 0:
                    u_tiles[c] = upool.tile([128, 512], f32, tag="u",
                                            name=f"u_{c}")
                ea = eapool.tile([128, 4, 256], f32, tag="ea",
                                 name=f"ea_{c}_{jbs[0]}")
                for t, jb in enumerate(jbs):
                    for m, (lt, rt) in enumerate(((l0_t, r0_t), (l1_t, r1_t))):
                        nc.tensor.matmul(
                            ea[:, 2 * t + m, :],
                            lhsT=lt[:, :, jb * 128:(jb + 1) * 128],
                            rhs=rt[:, :, c0:c0 + CHUNK],
                            start=True, stop=True,
                            perf_mode=mybir.MatmulPerfMode.DoubleRow,
                        )
                s_t = spool.tile([128, 4, 256], u8, tag="s",
                                 name=f"s_{c}_{jbs[0]}")
                eng = min(("act", "dve"), key=lambda e: t_eng[e] + COST[(e, nt)])
                t_eng[eng] += COST[(eng, nt)]
                if jbs[-1] == njb - 1:
                    t_eng["dve"] += 700.0  # epilogue PSUM-side ops land on DVE
                if eng == "act":
                    nc.scalar.activation(
                        s_t[:, 0:2 * nt, :].bitcast(mybir.dt.float8e4),
                        ea[:, 0:2 * nt, :],
                        mybir.ActivationFunctionType.Exp,
                    )
                else:
                    nc.vector.tensor_scalar(
                        s_t[:, 0:2 * nt, :], ea[:, 0:2 * nt, :], SCH_A, SCH_B,
                        mybir.AluOpType.mult, mybir.AluOpType.add,
                    )
                return (c, s_t, jbs, gi == 0, jbs[-1] == njb - 1)

            for c, gi in items:
                deferred.append(work(c, gi))
                if len(deferred) > DEFER:
                    emit_accums(deferred.pop(0))
            while deferred:
                emit_accums(deferred.pop(0))

    _split_sync_waits(nc)
    return nc


def _split3_e4(v):
    parts = []
    r = np.asarray(v, np.float32)
    for _ in range(3):
        p = r.astype(E4NP).astype(np.float32)
        parts.append(p)
        r = r - p
    return parts


def _build_features(gf, sigma, wv):
    """24 (L_row, R_row) pairs of e4m3 rows s.t. sum_r L_r[j]*R_r[i] =
    -d2[j,i]/(2 sigma^2) + ln(wv) + SCALE_OCT*ln2 to ~1e-2 abs."""
    gf = np.asarray(gf, np.float32)
    n = gf.shape[0]
    g = np.float32(1.0 / (2.0 * sigma * sigma))
    sq = (gf * gf).sum(axis=1)
    a = 2.0 * g * gf
    b = gf
    dh = -g * sq * 0.5
    ch = (-g * sq + np.float32(np.log(wv))
          + np.float32(SCALE_OCT * np.log(2.0))) * 0.5

    A = _split3_e4(a)
    B = _split3_e4(b)
    D = _split3_e4(dh)
    C = _split3_e4(ch)
    ones = np.ones(n, np.float32)
    twos = 2.0 * ones

    Lrows, Rrows = [], []
    for ka, kb in [(0, 0), (0, 1), (1, 0), (1, 1), (0, 2), (2, 0)]:
        for d in range(DG):
            Lrows.append(A[ka][:, d])
            Rrows.append(B[kb][:, d])
    for k in range(3):
        Lrows.append(D[k])
        Rrows.append(twos)
    for k in range(3):
        Lrows.append(twos)
        Rrows.append(C[k])
    assert len(Lrows) == 2 * KI
    L = np.zeros((KI, 2, n), E4NP)
    R = np.zeros((KI, 2, n), E4NP)
    for r in range(2 * KI):
        L[r % KI, r // KI] = Lrows[r].astype(E4NP)
        R[r % KI, r // KI] = Rrows[r].astype(E4NP)
    return L, R


def _prepare_inputs(gf0, gf1, node_v_feats, weights, sigmas, n_cores=NCORES):
    weights = np.asarray(weights, np.float32)
    sigmas = np.asarray(sigmas, np.float32)
    V = np.asarray(node_v_feats, np.float32)
    n = V.shape[0]
    ni = n // n_cores
    njb = n // 128

    L0, R0full = _build_features(gf0, float(sigmas[0]), float(weights[0]))
    L1, R1full = _build_features(gf1, float(sigmas[1]), float(weights[1]))

    vaug = np.concatenate(
        [V.astype(E4NP).astype(np.float32), np.ones((n, 1), np.float32)], axis=1
    ).astype(E4NP)                                    # [n, 129]
    va = np.zeros((128, 2 * njb, 129), E4NP)
    blocks = vaug.reshape(njb, 128, 129)
    for m in range(2):
        va[:, m::2, :] = blocks.transpose(1, 0, 2)

    in_maps = []
    for c in range(n_cores):
        rows = slice(c * ni, (c + 1) * ni)
        vr = np.ascontiguousarray(
            V[rows].reshape(ni // 128, 128, DV).transpose(1, 0, 2).reshape(128, ni)
        )
        in_maps.append({
            "L0": np.ascontiguousarray(L0),
            "L1": np.ascontiguousarray(L1),
            "R0": np.ascontiguousarray(R0full[:, :, rows]),
            "R1": np.ascontiguousarray(R1full[:, :, rows]),
            "VA": va,
            "VR": vr,
        })
    return in_maps


_NC_CACHE = {}


def _get_nc(n_j=N, n_i=NI):
    key = (n_j, n_i)
    if key not in _NC_CACHE:
        _NC_CACHE[key] = build_nc(n_j, n_i)
    return _NC_CACHE[key]


_EXEC_CACHE = {}


def _get_executor(nc, n_cores):
    key = (id(nc), n_cores)
    if key in _EXEC_CACHE:
        return _EXEC_CACHE[key]
    import jax
    from jax.experimental.shard_map import shard_map
    from jax.sharding import Mesh, PartitionSpec
    from concourse.bass2jax import (
        install_neuronx_cc_hook,
        _bass_exec_p,
        partition_id_tensor,
    )

    install_neuronx_cc_hook()

    partition_name = nc.partition_id_tensor.name if nc.partition_id_tensor else None
    in_names, out_names, out_avals = [], [], []
    for alloc in nc.m.functions[0].allocations:
        if not isinstance(alloc, mybir.MemoryLocationSet):
            continue
        name = alloc.memorylocations[0].name
        if alloc.kind == "ExternalInput":
            if name != partition_name:
                in_names.append(name)
        elif alloc.kind == "ExternalOutput":
            out_names.append(name)
            out_avals.append(
                jax.core.ShapedArray(tuple(alloc.tensor_shape), mybir.dt.np(alloc.dtype))
            )
    n_params = len(in_names)
    all_names = list(in_names) + list(out_names)
    if partition_name is not None:
        all_names.append(partition_name)

    def _body(*args):
        operands = list(args)
        if partition_name is not None:
            operands.append(partition_id_tensor())
        outs = _bass_exec_p.bind(
            *operands,
            out_avals=tuple(out_avals),
            in_names=tuple(all_names),
            out_names=tuple(out_names),
            lowering_input_output_aliases=(),
            sim_require_finite=True,
            sim_require_nnan=True,
            nc=nc,
        )
        return tuple(outs)

    devices = jax.devices()[:n_cores]
    mesh = Mesh(np.asarray(devices), ("core",))
    n_outs = len(out_names)
    replicated = frozenset(["L0", "L1", "VA"])
    in_specs = tuple(
        PartitionSpec() if name in replicated else PartitionSpec("core")
        for name in in_names
    ) + (PartitionSpec("core"),) * n_outs
    sharded = jax.jit(
        shard_map(
            _body,
            mesh=mesh,
            in_specs=in_specs,
            out_specs=(PartitionSpec("core"),) * n_outs,
            check_rep=False,
        ),
        donate_argnums=tuple(range(n_params, n_params + n_outs)),
        keep_unused=True,
    )
    entry = (sharded, in_names, out_names, out_avals, replicated)
    _EXEC_CACHE[key] = entry
    return entry


def _run(nc, in_maps, n_cores):
    sharded, in_names, out_names, out_avals, replicated = _get_executor(nc, n_cores)
    concat_in = [
        in_maps[0][name] if name in replicated
        else np.concatenate([in_maps[c][name] for c in range(n_cores)], axis=0)
        for name in in_names
    ]
    concat_zeros = [
        np.zeros((n_cores * a.shape[0], *a.shape[1:]), a.dtype) for a in out_avals
    ]
    out_arrs = sharded(*concat_in, *concat_zeros)
    return [
        {
            name: np.asarray(out_arrs[i]).reshape(n_cores, *out_avals[i].shape)[c]
            for i, name in enumerate(out_names)
        }
        for c in range(n_cores)
    ]


def kernel(gf0, gf1, node_v_feats, weights, sigmas):
    import jax

    in_maps = _prepare_inputs(gf0, gf1, node_v_feats, weights, sigmas)
    nc = _get_nc()
    last_exc = None
    for attempt in range(3):
        try:
            results = _run(nc, in_maps, NCORES)
            jax.effects_barrier()
            out = np.concatenate([results[c]["out"] for c in range(NCORES)], axis=0)
            return np.ascontiguousarray(out.astype(np.float32))
        except Exception as e:
            last_exc = e
            _EXEC_CACHE.clear()
            try:
                jax.clear_caches()
            except Exception:
                pass
            try:
                jax._src.xla_bridge.backends.cache_clear()  # type: ignore[attr-defined]
            except Exception:
                pass
            import time as _time
            _time.sleep(5 * (attempt + 1))
    raise last_exc


# revision 3
# speedup vs baseline: 1.0204x; 1.0069x over previous
"""Trainium2 Bass kernel v2 for nn_GAttn_67147518705771.

Computes: score = w0*RBF(gf0, s0) + w1*RBF(gf1, s1)  (N x N)
          attn  = score / (rowsum(score) + 0.01)
          out   = attn @ V + V

Row-parallel over 8 cores; core c owns output rows [c*1024, (c+1)*1024).

v2 strategy (vs the f16 baseline):
  * All matmuls are fp8e4 + DoubleRow (0.5 cyc/col, 256-row contraction):
    - E matmul per modality: 24 e4m3 feature rows as [12, 2, *] pairs
      produce E[j,i]*1 with ln(w) + 5*ln2 folded in (S scaled x32 so the
      e4m3 subnormal band sits harmlessly low; epilogue uses EPS*32).
    - Accumulation pairs the TWO MODALITY S tiles in one DoubleRow matmul
      against a byte-duplicated VA block [128, 2, 129] ([V|1] e4m3), so
      S0@VA + S1@VA costs 64.5 PE cycles per (jb, i128) block.
  * exp runs on ACT (native Exp -> fp8 out) and DVE (Schraudolph affine ->
    u8, bitcast e4m3). GPSIMD has no PSUM port so it only issues DMAs.
  * PSUM: 3 x [128,4,256] E tiles (2 banks each) + 2 x U accumulator banks.
"""

import numpy as np
import ml_dtypes

import concourse.bass as bass
import concourse.tile as tile
import concourse.mybir as mybir

E4NP = ml_dtypes.float8_e4m3
EPS = 0.01
N = 8192
DG = 3
DV = 128
NCORES = 8
NI = N // NCORES
KI = 12            # feature pair-rows per modality (24 rows via DoubleRow)
CHUNK = 256
NJB = N // 128
SCALE_OCT = 5      # S scaled by 2^5; epilogue divides by (rowsum + EPS*32)
EPS_S = EPS * (1 << SCALE_OCT)
SCH_A = float(np.float32(8.0 * 1.4426950408889634))
SCH_B = float(np.float32(55.537))
DEFER = 3          # groups between exp emission and its accum matmuls

# per-group exp cost estimates (ns) for greedy ACT/DVE balancing
COST = {
    ("act", 1): 612.0, ("act", 2): 1038.0,
    ("dve", 1): 658.0, ("dve", 2): 1192.0,
}


def _split_sync_waits(nc, maxw=1):
    """Walrus rejects instructions with >1 sync waits. Hoist extras onto
    single-wait InstNoOp carriers (same engine). The kernel-tail SP drain's
    waits are spread round-robin across all engines (barrier follows)."""
    n_split = n_carriers = 0
    eng_rr = [
        mybir.EngineType.SP,
        mybir.EngineType.Activation,
        mybir.EngineType.DVE,
        mybir.EngineType.PE,
        mybir.EngineType.Pool,
    ]
    for f in nc.m.functions:
        for bb in f.blocks:
            insts = list(bb.instructions)
            out, changed = [], False
            for inst in insts:
                si = inst.sync_info
                waits = list(si.on_wait) if si and si.on_wait else []
                if len(waits) > maxw:
                    n_split += 1
                    changed = True
                    is_tail_drain = (
                        isinstance(inst, mybir.InstDrain)
                        and inst.engine == mybir.EngineType.SP
                        and len(waits) > 2
                    )
                    for k, w in enumerate(waits[:-maxw]):
                        nop = mybir.InstNoOp(name=f"waitnop-{n_carriers}", ins=[], outs=[])
                        n_carriers += 1
                        nop.engine = eng_rr[k % len(eng_rr)] if is_tail_drain else inst.engine
                        nop.sync_info = mybir.SyncInfo(on_wait=[w], on_update=[])
                        out.append(nop)
                    inst.sync_info = mybir.SyncInfo(
                        on_wait=waits[-maxw:], on_update=list(si.on_update or [])
                    )
                out.append(inst)
            if changed:
                bb.instructions = out
    return n_split, n_carriers


def build_nc(n_j=N, n_i=NI):
    f32 = mybir.dt.float32
    fp8 = mybir.dt.float8e4
    u8 = mybir.dt.uint8
    njb = n_j // 128
    nchunks = n_i // CHUNK

    nc = bass.Bass("TRN2", target_bir_lowering=False, debug=False)
    L0 = nc.dram_tensor("L0", [KI, 2, n_j], fp8, kind="ExternalInput").ap()
    L1 = nc.dram_tensor("L1", [KI, 2, n_j], fp8, kind="ExternalInput").ap()
    R0 = nc.dram_tensor("R0", [KI, 2, n_i], fp8, kind="ExternalInput").ap()
    R1 = nc.dram_tensor("R1", [KI, 2, n_i], fp8, kind="ExternalInput").ap()
    # VA[p, 2*jb+m, v] = [V|1] row (jb*128+p), col v — duplicated per pair m.
    VA = nc.dram_tensor("VA", [128, 2 * njb, 129], fp8, kind="ExternalInput").ap()
    VR = nc.dram_tensor("VR", [128, n_i], f32, kind="ExternalInput").ap()
    OUT = nc.dram_tensor("out", [n_i, DV], f32, kind="ExternalOutput").ap()

    # per-chunk groups: first chunk leads with a single j-block so the first
    # exp starts early; last chunk ends with singles for a short tail drain.
    g_first = [[0]] + [[j, j + 1] for j in range(1, njb - 1, 2)] + [[njb - 1]]
    g_mid = [[j, j + 1] for j in range(0, njb, 2)]
    g_last = [[j, j + 1] for j in range(0, njb - 2, 2)] + [[njb - 2], [njb - 1]]

    with tile.TileContext(nc) as tc:
        with (
            tc.tile_pool(name="resident", bufs=1) as rpool,
            tc.tile_pool(name="eapool", bufs=3, space="PSUM") as eapool,
            tc.tile_pool(name="upool", bufs=2, space="PSUM") as upool,
            tc.tile_pool(name="spool", bufs=6) as spool,
            tc.tile_pool(name="opool", bufs=4) as opool,
            tc.tile_pool(name="scalars", bufs=4) as scpool,
        ):
            # --- resident inputs, spread across DMA-issue rails ---
            l0_t = rpool.tile([KI, 2, n_j], fp8)
            l1_t = rpool.tile([KI, 2, n_j], fp8)
            r0_t = rpool.tile([KI, 2, n_i], fp8)
            r1_t = rpool.tile([KI, 2, n_i], fp8)
            va_t = rpool.tile([128, 2 * njb, 129], fp8)
            vr_t = rpool.tile([128, n_i], f32)

            # Modality-0 inputs ride sync/HWDGE, modality-1 rides the Pool
            # SWDGE path so their configs run in parallel (HWDGE configs
            # serialize at ~625ns on one shared device); ACT issues no DMAs so
            # its sequencer reaches the first exp immediately.
            PIECE = 8  # j-blocks per L piece
            nc.sync.dma_start(r0_t[:], R0)
            nc.sync.dma_start(l0_t[:, :, 0:PIECE * 128], L0[:, :, 0:PIECE * 128])
            nc.gpsimd.dma_start(r1_t[:], R1)
            nc.gpsimd.dma_start(l1_t[:, :, 0:PIECE * 128], L1[:, :, 0:PIECE * 128])
            nc.gpsimd.dma_start(va_t[:, 0:2 * PIECE, :], VA[:, 0:2 * PIECE, :])
            for p in range(PIECE, njb, PIECE):
                nc.sync.dma_start(
                    l0_t[:, :, p * 128:(p + PIECE) * 128],
                    L0[:, :, p * 128:(p + PIECE) * 128],
                )
                nc.sync.dma_start(
                    l1_t[:, :, p * 128:(p + PIECE) * 128],
                    L1[:, :, p * 128:(p + PIECE) * 128],
                )
                nc.gpsimd.dma_start(
                    va_t[:, 2 * p:2 * (p + PIECE), :], VA[:, 2 * p:2 * (p + PIECE), :]
                )
            nc.sync.dma_start(vr_t[:], VR)

            # ACT exp-table preload + PE p-state warm-up during input DMA.
            dummy = scpool.tile([128, 1], f32, tag="dummy")
            nc.vector.memset(dummy[:], 0.0)
            dummy2 = scpool.tile([128, 1], f32, tag="dummy2")
            nc.scalar.activation(dummy2[:], dummy[:], mybir.ActivationFunctionType.Exp)
            dmm = scpool.tile([1, 256], mybir.dt.bfloat16, tag="dmm")
            nc.vector.memset(dmm[:], 0.0)
            e_warm = eapool.tile([128, 4, 256], f32, tag="ea", name="e_warm")
            for _ in range(14):
                nc.tensor.matmul(
                    e_warm[:, 0, :], lhsT=dmm[:, 0:128], rhs=dmm[:, 0:256],
                    start=True, stop=True,
                )

            # --- global stream of (chunk, group) items with greedy ACT/DVE
            # balance; accum matmuls deferred DEFER groups so the PE never
            # stalls on an in-flight exp, including across chunk boundaries.
            chunk_groups = [g_first] + [g_mid] * (nchunks - 2) + [g_last]
            items = [(c, gi) for c in range(nchunks)
                     for gi in range(len(chunk_groups[c]))]
            t_eng = {"act": 0.0, "dve": 0.0}
            u_tiles = {}
            deferred = []

            def emit_accums(item):
                c, s_t, jbs, first, last = item
                u_t = u_tiles[c]
                for t, jb in enumerate(jbs):
                    for isub in range(2):
                        # DR psum writes need >=1KB-aligned offsets: slots at
                        # f32 cols 0 and 256 of a full bank.
                        nc.tensor.matmul(
                            u_t[:, isub * 256:isub * 256 + 129],
                            lhsT=s_t[:, 2 * t:2 * t + 2,
                                     isub * 128:(isub + 1) * 128].bitcast(
                                         mybir.dt.float8e4),
                            rhs=va_t[:, 2 * jb:2 * jb + 2, :],
                            start=(first and t == 0 and isub == 0),
                            stop=(last and t == len(jbs) - 1 and isub == 1),
                            skip_group_check=True,
                            perf_mode=mybir.MatmulPerfMode.DoubleRow,
                        )
                if last:
                    emit_epilogue(c)

            def emit_epilogue(c):
                # out rows = U/(rowsum+EPS_S) + V residual; residual add and
                # store run on Pool/SP rails, PSUM-side ops on DVE.
                u_t = u_tiles.pop(c)
                for isub in range(2):
                    g = c * 2 + isub
                    ub = u_t[:, isub * 256:isub * 256 + 129]
                    rt = scpool.tile([128, 1], f32, tag="rt", name=f"rt_{g}")
                    nc.vector.tensor_scalar_add(rt[:], ub[:, 128:129], EPS_S)
                    ri = scpool.tile([128, 1], f32, tag="ri", name=f"ri_{g}")
                    nc.vector.reciprocal(ri[:], rt[:])
                    ot = opool.tile([128, DV], f32, tag="ot", name=f"ot_{g}")
                    nc.vector.tensor_scalar_mul(ot[:], ub[:, 0:DV], ri[:])
                    nc.gpsimd.tensor_add(ot[:], ot[:],
                                         vr_t[:, g * 128:(g + 1) * 128])
                    out_eng = (nc.sync if (isub == 0 or c == nchunks - 1)
                               else nc.gpsimd)
                    out_eng.dma_start(OUT[g * 128:(g + 1) * 128, :], ot[:])

            def work(c, gi):
                jbs = chunk_groups[c][gi]
                nt = len(jbs)
                c0 = c * CHUNK
                if gi == 0:
                    u_tiles[c] = upool.tile([128, 512], f32, tag="u",
                                            name=f"u_{c}")
                ea = eapool.tile([128, 4, 256], f32, tag="ea",
                                 name=f"ea_{c}_{jbs[0]}")
                for t, jb in enumerate(jbs):
                    for m, (lt, rt) in enumerate(((l0_t, r0_t), (l1_t, r1_t))):
                        nc.tensor.matmul(
                            ea[:, 2 * t + m, :],
                            lhsT=lt[:, :, jb * 128:(jb + 1) * 128],
                            rhs=rt[:, :, c0:c0 + CHUNK],
                            start=True, stop=True,
                            perf_mode=mybir.MatmulPerfMode.DoubleRow,
                        )
                s_t = spool.tile([128, 4, 256], u8, tag="s",
                                 name=f"s_{c}_{jbs[0]}")
                is_final = (c == nchunks - 1 and gi >= len(chunk_groups[c]) - 1)
                eng = ("act" if is_final else
                       min(("act", "dve"), key=lambda e: t_eng[e] + COST[(e, nt)]))
                t_eng[eng] += COST[(eng, nt)]
                if jbs[-1] == njb - 1:
                    t_eng["dve"] += 700.0  # epilogue PSUM-side ops land on DVE
                if eng == "act":
                    nc.scalar.activation(
                        s_t[:, 0:2 * nt, :].bitcast(mybir.dt.float8e4),
                        ea[:, 0:2 * nt, :],
                        mybir.ActivationFunctionType.Exp,
                    )
                else:
                    nc.vector.tensor_scalar(
                        s_t[:, 0:2 * nt, :], ea[:, 0:2 * nt, :], SCH_A, SCH_B,
                        mybir.AluOpType.mult, mybir.AluOpType.add,
                    )
                return (c, s_t, jbs, gi == 0, jbs[-1] == njb - 1)

            for c, gi in items:
                deferred.append(work(c, gi))
                if len(deferred) > DEFER:
                    emit_accums(deferred.pop(0))
            while deferred:
                emit_accums(deferred.pop(0))

    _split_sync_waits(nc)
    return nc


def _split3_e4(v):
    parts = []
    r = np.asarray(v, np.float32)
    for _ in range(3):
        p = r.astype(E4NP).astype(np.float32)
        parts.append(p)
        r = r - p
    return parts


def _build_features(gf, sigma, wv):
    """24 (L_row, R_row) pairs of e4m3 rows s.t. sum_r L_r[j]*R_r[i] =
    -d2[j,i]/(2 sigma^2) + ln(wv) + SCALE_OCT*ln2 to ~1e-2 abs."""
    gf = np.asarray(gf, np.float32)
    n = gf.shape[0]
    g = np.float32(1.0 / (2.0 * sigma * sigma))
    sq = (gf * gf).sum(axis=1)
    a = 2.0 * g * gf
    b = gf
    dh = -g * sq * 0.5
    ch = (-g * sq + np.float32(np.log(wv))
          + np.float32(SCALE_OCT * np.log(2.0))) * 0.5

    A = _split3_e4(a)
    B = _split3_e4(b)
    D = _split3_e4(dh)
    C = _split3_e4(ch)
    ones = np.ones(n, np.float32)
    twos = 2.0 * ones

    Lrows, Rrows = [], []
    for ka, kb in [(0, 0), (0, 1), (1, 0), (1, 1), (0, 2), (2, 0)]:
        for d in range(DG):
            Lrows.append(A[ka][:, d])
            Rrows.append(B[kb][:, d])
    for k in range(3):
        Lrows.append(D[k])
        Rrows.append(twos)
    for k in range(3):
        Lrows.append(twos)
        Rrows.append(C[k])
    assert len(Lrows) == 2 * KI
    L = np.zeros((KI, 2, n), E4NP)
    R = np.zeros((KI, 2, n), E4NP)
    for r in range(2 * KI):
        L[r % KI, r // KI] = Lrows[r].astype(E4NP)
        R[r % KI, r // KI] = Rrows[r].astype(E4NP)
    return L, R


def _prepare_inputs(gf0, gf1, node_v_feats, weights, sigmas, n_cores=NCORES):
    weights = np.asarray(weights, np.float32)
    sigmas = np.asarray(sigmas, np.float32)
    V = np.asarray(node_v_feats, np.float32)
    n = V.shape[0]
    ni = n // n_cores
    njb = n // 128

    L0, R0full = _build_features(gf0, float(sigmas[0]), float(weights[0]))
    L1, R1full = _build_features(gf1, float(sigmas[1]), float(weights[1]))

    vaug = np.concatenate(
        [V.astype(E4NP).astype(np.float32), np.ones((n, 1), np.float32)], axis=1
    ).astype(E4NP)                                    # [n, 129]
    va = np.zeros((128, 2 * njb, 129), E4NP)
    blocks = vaug.reshape(njb, 128, 129)
    for m in range(2):
        va[:, m::2, :] = blocks.transpose(1, 0, 2)

    in_maps = []
    for c in range(n_cores):
        rows = slice(c * ni, (c + 1) * ni)
        vr = np.ascontiguousarray(
            V[rows].reshape(ni // 128, 128, DV).transpose(1, 0, 2).reshape(128, ni)
        )
        in_maps.append({
            "L0": np.ascontiguousarray(L0),
            "L1": np.ascontiguousarray(L1),
            "R0": np.ascontiguousarray(R0full[:, :, rows]),
            "R1": np.ascontiguousarray(R1full[:, :, rows]),
            "VA": va,
            "VR": vr,
        })
    return in_maps


_NC_CACHE = {}


def _get_nc(n_j=N, n_i=NI):
    key = (n_j, n_i)
    if key not in _NC_CACHE:
        _NC_CACHE[key] = build_nc(n_j, n_i)
    return _NC_CACHE[key]


_EXEC_CACHE = {}


def _get_executor(nc, n_cores):
    key = (id(nc), n_cores)
    if key in _EXEC_CACHE:
        return _EXEC_CACHE[key]
    import jax
    from jax.experimental.shard_map import shard_map
    from jax.sharding import Mesh, PartitionSpec
    from concourse.bass2jax import (
        install_neuronx_cc_hook,
        _bass_exec_p,
        partition_id_tensor,
    )

    install_neuronx_cc_hook()

    partition_name = nc.partition_id_tensor.name if nc.partition_id_tensor else None
    in_names, out_names, out_avals = [], [], []
    for alloc in nc.m.functions[0].allocations:
        if not isinstance(alloc, mybir.MemoryLocationSet):
            continue
        name = alloc.memorylocations[0].name
        if alloc.kind == "ExternalInput":
            if name != partition_name:
                in_names.append(name)
        elif alloc.kind == "ExternalOutput":
            out_names.append(name)
            out_avals.append(
                jax.core.ShapedArray(tuple(alloc.tensor_shape), mybir.dt.np(alloc.dtype))
            )
    n_params = len(in_names)
    all_names = list(in_names) + list(out_names)
    if partition_name is not None:
        all_names.append(partition_name)

    def _body(*args):
        operands = list(args)
        if partition_name is not None:
            operands.append(partition_id_tensor())
        outs = _bass_exec_p.bind(
            *operands,
            out_avals=tuple(out_avals),
            in_names=tuple(all_names),
            out_names=tuple(out_names),
            lowering_input_output_aliases=(),
            sim_require_finite=True,
            sim_require_nnan=True,
            nc=nc,
        )
        return tuple(outs)

    devices = jax.devices()[:n_cores]
    mesh = Mesh(np.asarray(devices), ("core",))
    n_outs = len(out_names)
    replicated = frozenset(["L0", "L1", "VA"])
    in_specs = tuple(
        PartitionSpec() if name in replicated else PartitionSpec("core")
        for name in in_names
    ) + (PartitionSpec("core"),) * n_outs
    sharded = jax.jit(
        shard_map(
            _body,
            mesh=mesh,
            in_specs=in_specs,
            out_specs=(PartitionSpec("core"),) * n_outs,
            check_rep=False,
        ),
        donate_argnums=tuple(range(n_params, n_params + n_outs)),
        keep_unused=True,
    )
    entry = (sharded, in_names, out_names, out_avals, replicated)
    _EXEC_CACHE[key] = entry
    return entry


def _run(nc, in_maps, n_cores):
    sharded, in_names, out_names, out_avals, replicated = _get_executor(nc, n_cores)
    concat_in = [
        in_maps[0][name] if name in replicated
        else np.concatenate([in_maps[c][name] for c in range(n_cores)], axis=0)
        for name in in_names
    ]
    concat_zeros = [
        np.zeros((n_cores * a.shape[0], *a.shape[1:]), a.dtype) for a in out_avals
    ]
    out_arrs = sharded(*concat_in, *concat_zeros)
    return [
        {
            name: np.asarray(out_arrs[i]).reshape(n_cores, *out_avals[i].shape)[c]
            for i, name in enumerate(out_names)
        }
        for c in range(n_cores)
    ]


def kernel(gf0, gf1, node_v_feats, weights, sigmas):
    import jax

    in_maps = _prepare_inputs(gf0, gf1, node_v_feats, weights, sigmas)
    nc = _get_nc()
    last_exc = None
    for attempt in range(3):
        try:
            results = _run(nc, in_maps, NCORES)
            jax.effects_barrier()
            out = np.concatenate([results[c]["out"] for c in range(NCORES)], axis=0)
            return np.ascontiguousarray(out.astype(np.float32))
        except Exception as e:
            last_exc = e
            _EXEC_CACHE.clear()
            try:
                jax.clear_caches()
            except Exception:
                pass
            try:
                jax._src.xla_bridge.backends.cache_clear()  # type: ignore[attr-defined]
            except Exception:
                pass
            import time as _time
            _time.sleep(5 * (attempt + 1))
    raise last_exc


# revision 4
# speedup vs baseline: 1.0245x; 1.0040x over previous
"""Trainium2 Bass kernel v2 for nn_GAttn_67147518705771.

Computes: score = w0*RBF(gf0, s0) + w1*RBF(gf1, s1)  (N x N)
          attn  = score / (rowsum(score) + 0.01)
          out   = attn @ V + V

Row-parallel over 8 cores; core c owns output rows [c*1024, (c+1)*1024).

v2 strategy (vs the f16 baseline):
  * All matmuls are fp8e4 + DoubleRow (0.5 cyc/col, 256-row contraction):
    - E matmul per modality: 24 e4m3 feature rows as [12, 2, *] pairs
      produce E[j,i]*1 with ln(w) + 5*ln2 folded in (S scaled x32 so the
      e4m3 subnormal band sits harmlessly low; epilogue uses EPS*32).
    - Accumulation pairs the TWO MODALITY S tiles in one DoubleRow matmul
      against a byte-duplicated VA block [128, 2, 129] ([V|1] e4m3), so
      S0@VA + S1@VA costs 64.5 PE cycles per (jb, i128) block.
  * exp runs on ACT (native Exp -> fp8 out) and DVE (Schraudolph affine ->
    u8, bitcast e4m3). GPSIMD has no PSUM port so it only issues DMAs.
  * PSUM: 3 x [128,4,256] E tiles (2 banks each) + 2 x U accumulator banks.
"""

import numpy as np
import ml_dtypes

import concourse.bass as bass
import concourse.tile as tile
import concourse.mybir as mybir

E4NP = ml_dtypes.float8_e4m3
EPS = 0.01
N = 8192
DG = 3
DV = 128
NCORES = 8
NI = N // NCORES
KI = 12            # feature pair-rows per modality (24 rows via DoubleRow)
CHUNK = 256
NJB = N // 128
SCALE_OCT = 5      # S scaled by 2^5; epilogue divides by (rowsum + EPS*32)
EPS_S = EPS * (1 << SCALE_OCT)
SCH_A = float(np.float32(8.0 * 1.4426950408889634))
SCH_B = float(np.float32(55.537))
DEFER = 3          # groups between exp emission and its accum matmuls

# per-group exp cost estimates (ns) for greedy ACT/DVE balancing
COST = {
    ("act", 1): 612.0, ("act", 2): 1038.0,
    ("dve", 1): 658.0, ("dve", 2): 1192.0,
}


def _split_sync_waits(nc, maxw=1):
    """Walrus rejects instructions with >1 sync waits. Hoist extras onto
    single-wait InstNoOp carriers (same engine). The kernel-tail SP drain's
    waits are spread round-robin across all engines (barrier follows)."""
    n_split = n_carriers = 0
    eng_rr = [
        mybir.EngineType.SP,
        mybir.EngineType.Activation,
        mybir.EngineType.DVE,
        mybir.EngineType.PE,
        mybir.EngineType.Pool,
    ]
    for f in nc.m.functions:
        for bb in f.blocks:
            insts = list(bb.instructions)
            out, changed = [], False
            for inst in insts:
                si = inst.sync_info
                waits = list(si.on_wait) if si and si.on_wait else []
                if len(waits) > maxw:
                    n_split += 1
                    changed = True
                    is_tail_drain = (
                        isinstance(inst, mybir.InstDrain)
                        and inst.engine == mybir.EngineType.SP
                        and len(waits) > 2
                    )
                    for k, w in enumerate(waits[:-maxw]):
                        nop = mybir.InstNoOp(name=f"waitnop-{n_carriers}", ins=[], outs=[])
                        n_carriers += 1
                        nop.engine = eng_rr[k % len(eng_rr)] if is_tail_drain else inst.engine
                        nop.sync_info = mybir.SyncInfo(on_wait=[w], on_update=[])
                        out.append(nop)
                    inst.sync_info = mybir.SyncInfo(
                        on_wait=waits[-maxw:], on_update=list(si.on_update or [])
                    )
                out.append(inst)
            if changed:
                bb.instructions = out
    return n_split, n_carriers


def build_nc(n_j=N, n_i=NI):
    f32 = mybir.dt.float32
    fp8 = mybir.dt.float8e4
    u8 = mybir.dt.uint8
    njb = n_j // 128
    nchunks = n_i // CHUNK

    nc = bass.Bass("TRN2", target_bir_lowering=False, debug=False)
    L0 = nc.dram_tensor("L0", [KI, 2, n_j], fp8, kind="ExternalInput").ap()
    L1 = nc.dram_tensor("L1", [KI, 2, n_j], fp8, kind="ExternalInput").ap()
    R0 = nc.dram_tensor("R0", [KI, 2, n_i], fp8, kind="ExternalInput").ap()
    R1 = nc.dram_tensor("R1", [KI, 2, n_i], fp8, kind="ExternalInput").ap()
    # VA[p, 2*jb+m, v] = [V|1] row (jb*128+p), col v — duplicated per pair m.
    VA = nc.dram_tensor("VA", [128, 2 * njb, 129], fp8, kind="ExternalInput").ap()
    VR = nc.dram_tensor("VR", [128, n_i], f32, kind="ExternalInput").ap()
    # Pool-lane operands: j-side scalars (replicated across cores) and i-side
    # replicated rows (per-core) for computing E on GPSIMD entirely in SBUF.
    AS = [nc.dram_tensor(f"AS{m}", [128, 4 * njb], f32, kind="ExternalInput").ap()
          for m in range(2)]
    BR = [[nc.dram_tensor(f"BR{m}{d}", [128, n_i], mybir.dt.float16,
                          kind="ExternalInput").ap() for d in range(3)]
          for m in range(2)]
    CR = [nc.dram_tensor(f"CR{m}", [128, n_i], f32, kind="ExternalInput").ap()
          for m in range(2)]
    OUT = nc.dram_tensor("out", [n_i, DV], f32, kind="ExternalOutput").ap()

    # per-chunk groups: first chunk leads with a single j-block so the first
    # exp starts early; last chunk ends with singles for a short tail drain.
    # POOL_JBS are computed by the GPSIMD lane instead (late in chunk 0 so its
    # operand DMAs have landed).
    POOL_JBS = {0: [57], 1: [9, 41], 2: [9, 41], 3: [9, 41]}
    POOL_JBS = {c: POOL_JBS.get(c, []) for c in range(nchunks)}

    def mk_groups(c, first_single, last_singles):
        rem = [j for j in range(njb) if j not in POOL_JBS[c]]
        gs = []
        if first_single:
            gs.append([rem.pop(0)])
        tail = [rem.pop(), rem.pop()] if last_singles else []
        while len(rem) >= 2:
            gs.append([rem.pop(0), rem.pop(0)])
        if rem:
            gs.append([rem.pop(0)])
        for j in sorted(tail):
            gs.append([j])
        return gs

    with tile.TileContext(nc) as tc:
        with (
            tc.tile_pool(name="resident", bufs=1) as rpool,
            tc.tile_pool(name="eapool", bufs=3, space="PSUM") as eapool,
            tc.tile_pool(name="upool", bufs=2, space="PSUM") as upool,
            tc.tile_pool(name="spool", bufs=6) as spool,
            tc.tile_pool(name="opool", bufs=4) as opool,
            tc.tile_pool(name="scalars", bufs=4) as scpool,
        ):
            # --- resident inputs, spread across DMA-issue rails ---
            l0_t = rpool.tile([KI, 2, n_j], fp8)
            l1_t = rpool.tile([KI, 2, n_j], fp8)
            r0_t = rpool.tile([KI, 2, n_i], fp8)
            r1_t = rpool.tile([KI, 2, n_i], fp8)
            va_t = rpool.tile([128, 2 * njb, 129], fp8)
            vr_t = rpool.tile([128, n_i], f32)
            as_t = [rpool.tile([128, 4 * njb], f32, name=f"as{m}")
                    for m in range(2)]
            br_t = [[rpool.tile([128, n_i], mybir.dt.float16, name=f"br{m}{d}")
                     for d in range(3)] for m in range(2)]
            cr_t = [rpool.tile([128, n_i], f32, name=f"cr{m}") for m in range(2)]

            # Modality-0 inputs ride sync/HWDGE, modality-1 rides the Pool
            # SWDGE path so their configs run in parallel (HWDGE configs
            # serialize at ~625ns on one shared device); ACT issues no DMAs so
            # its sequencer reaches the first exp immediately.
            PIECE = 8  # j-blocks per L piece
            nc.sync.dma_start(r0_t[:], R0)
            nc.sync.dma_start(l0_t[:, :, 0:PIECE * 128], L0[:, :, 0:PIECE * 128])
            nc.gpsimd.dma_start(r1_t[:], R1)
            nc.gpsimd.dma_start(l1_t[:, :, 0:PIECE * 128], L1[:, :, 0:PIECE * 128])
            nc.gpsimd.dma_start(va_t[:, 0:2 * PIECE, :], VA[:, 0:2 * PIECE, :])
            for p in range(PIECE, njb, PIECE):
                nc.sync.dma_start(
                    l0_t[:, :, p * 128:(p + PIECE) * 128],
                    L0[:, :, p * 128:(p + PIECE) * 128],
                )
                nc.sync.dma_start(
                    l1_t[:, :, p * 128:(p + PIECE) * 128],
                    L1[:, :, p * 128:(p + PIECE) * 128],
                )
                nc.gpsimd.dma_start(
                    va_t[:, 2 * p:2 * (p + PIECE), :], VA[:, 2 * p:2 * (p + PIECE), :]
                )
            for m in range(2):
                nc.sync.dma_start(as_t[m][:], AS[m])
                nc.sync.dma_start(cr_t[m][:], CR[m])
                for d in range(3):
                    nc.sync.dma_start(br_t[m][d][:], BR[m][d])
            nc.sync.dma_start(vr_t[:], VR)

            # ACT exp-table preload + PE p-state warm-up during input DMA.
            dummy = scpool.tile([128, 1], f32, tag="dummy")
            nc.vector.memset(dummy[:], 0.0)
            dummy2 = scpool.tile([128, 1], f32, tag="dummy2")
            nc.scalar.activation(dummy2[:], dummy[:], mybir.ActivationFunctionType.Exp)
            dmm = scpool.tile([1, 256], mybir.dt.bfloat16, tag="dmm")
            nc.vector.memset(dmm[:], 0.0)
            e_warm = eapool.tile([128, 4, 256], f32, tag="ea", name="e_warm")
            for _ in range(14):
                nc.tensor.matmul(
                    e_warm[:, 0, :], lhsT=dmm[:, 0:128], rhs=dmm[:, 0:256],
                    start=True, stop=True,
                )

            # --- global stream of (chunk, group) items with greedy ACT/DVE
            # balance; accum matmuls deferred DEFER groups so the PE never
            # stalls on an in-flight exp, including across chunk boundaries.
            chunk_groups = [mk_groups(0, True, False)] + [
                mk_groups(c, False, False) for c in range(1, nchunks - 1)
            ] + [mk_groups(nchunks - 1, False, True)]
            items = [(c, gi) for c in range(nchunks)
                     for gi in range(len(chunk_groups[c]))]
            t_eng = {"act": 0.0, "dve": 0.0}
            u_tiles = {}
            deferred = []
            pool_s = {}   # (c, jb) -> (s0, s1) u8 tiles

            def emit_pool_chain(c, jb):
                c0 = c * CHUNK
                s01 = []
                for m in range(2):
                    e = opool.tile([128, CHUNK], f32, tag="pe",
                                   name=f"pe_{c}_{jb}_{m}")
                    t = opool.tile([128, CHUNK], f32, tag="pt",
                                   name=f"pt_{c}_{jb}_{m}")
                    nc.gpsimd.tensor_scalar(
                        e[:], br_t[m][0][:, c0:c0 + CHUNK],
                        as_t[m][:, 0 * njb + jb:0 * njb + jb + 1],
                        as_t[m][:, 3 * njb + jb:3 * njb + jb + 1],
                        mybir.AluOpType.mult, mybir.AluOpType.add)
                    for d in (1, 2):
                        nc.gpsimd.tensor_scalar(
                            t[:], br_t[m][d][:, c0:c0 + CHUNK],
                            as_t[m][:, d * njb + jb:d * njb + jb + 1], None,
                            mybir.AluOpType.mult)
                        nc.gpsimd.tensor_add(e[:], e[:], t[:])
                    nc.gpsimd.tensor_add(e[:], e[:], cr_t[m][:, c0:c0 + CHUNK])
                    s_m = spool.tile([128, CHUNK], u8, tag=f"ps{m}",
                                     name=f"ps_{c}_{jb}_{m}")
                    nc.gpsimd.tensor_scalar(
                        s_m[:], e[:], SCH_A, SCH_B,
                        mybir.AluOpType.mult, mybir.AluOpType.add)
                    s01.append(s_m)
                pool_s[(c, jb)] = s01

            def emit_pool_accums(c):
                # plain (non-DoubleRow) fp8 matmuls; PE has ample slack
                u_t = u_tiles[c]
                for jb in POOL_JBS[c]:
                    s0, s1 = pool_s.pop((c, jb))
                    for isub in range(2):
                        for m, s_m in ((0, s0), (1, s1)):
                            nc.tensor.matmul(
                                u_t[:, isub * 256:isub * 256 + 129],
                                lhsT=s_m[:, isub * 128:(isub + 1) * 128].bitcast(
                                    mybir.dt.float8e4),
                                rhs=va_t[:, 2 * jb + m:2 * jb + m + 1, :],
                                start=False, stop=False,
                                skip_group_check=True,
                            )

            def emit_accums(item):
                c, s_t, jbs, first, last = item
                if last:
                    emit_pool_accums(c)
                u_t = u_tiles[c]
                for t, jb in enumerate(jbs):
                    for isub in range(2):
                        # DR psum writes need >=1KB-aligned offsets: slots at
                        # f32 cols 0 and 256 of a full bank.
                        nc.tensor.matmul(
                            u_t[:, isub * 256:isub * 256 + 129],
                            lhsT=s_t[:, 2 * t:2 * t + 2,
                                     isub * 128:(isub + 1) * 128].bitcast(
                                         mybir.dt.float8e4),
                            rhs=va_t[:, 2 * jb:2 * jb + 2, :],
                            start=(first and t == 0 and isub == 0),
                            stop=(last and t == len(jbs) - 1 and isub == 1),
                            skip_group_check=True,
                            perf_mode=mybir.MatmulPerfMode.DoubleRow,
                        )
                if last:
                    emit_epilogue(c)

            def emit_epilogue(c):
                # out rows = U/(rowsum+EPS_S) + V residual; residual add and
                # store run on Pool/SP rails, PSUM-side ops on DVE.
                u_t = u_tiles.pop(c)
                for isub in range(2):
                    g = c * 2 + isub
                    ub = u_t[:, isub * 256:isub * 256 + 129]
                    rt = scpool.tile([128, 1], f32, tag="rt", name=f"rt_{g}")
                    nc.vector.tensor_scalar_add(rt[:], ub[:, 128:129], EPS_S)
                    ri = scpool.tile([128, 1], f32, tag="ri", name=f"ri_{g}")
                    nc.vector.reciprocal(ri[:], rt[:])
                    ot = opool.tile([128, DV], f32, tag="ot", name=f"ot_{g}")
                    nc.vector.tensor_scalar_mul(ot[:], ub[:, 0:DV], ri[:])
                    nc.gpsimd.tensor_add(ot[:], ot[:],
                                         vr_t[:, g * 128:(g + 1) * 128])
                    out_eng = (nc.sync if (isub == 0 or c == nchunks - 1)
                               else nc.gpsimd)
                    out_eng.dma_start(OUT[g * 128:(g + 1) * 128, :], ot[:])

            def work(c, gi):
                jbs = chunk_groups[c][gi]
                nt = len(jbs)
                c0 = c * CHUNK
                if gi == 0:
                    u_tiles[c] = upool.tile([128, 512], f32, tag="u",
                                            name=f"u_{c}")
                    for jb in POOL_JBS[c]:
                        emit_pool_chain(c, jb)
                ea = eapool.tile([128, 4, 256], f32, tag="ea",
                                 name=f"ea_{c}_{jbs[0]}")
                for t, jb in enumerate(jbs):
                    for m, (lt, rt) in enumerate(((l0_t, r0_t), (l1_t, r1_t))):
                        nc.tensor.matmul(
                            ea[:, 2 * t + m, :],
                            lhsT=lt[:, :, jb * 128:(jb + 1) * 128],
                            rhs=rt[:, :, c0:c0 + CHUNK],
                            start=True, stop=True,
                            perf_mode=mybir.MatmulPerfMode.DoubleRow,
                        )
                s_t = spool.tile([128, 4, 256], u8, tag="s",
                                 name=f"s_{c}_{jbs[0]}")
                is_final = (c == nchunks - 1 and gi >= len(chunk_groups[c]) - 1)
                eng = ("act" if is_final else
                       min(("act", "dve"), key=lambda e: t_eng[e] + COST[(e, nt)]))
                t_eng[eng] += COST[(eng, nt)]
                if jbs[-1] == njb - 1:
                    t_eng["dve"] += 700.0  # epilogue PSUM-side ops land on DVE
                if eng == "act":
                    nc.scalar.activation(
                        s_t[:, 0:2 * nt, :].bitcast(mybir.dt.float8e4),
                        ea[:, 0:2 * nt, :],
                        mybir.ActivationFunctionType.Exp,
                    )
                else:
                    nc.vector.tensor_scalar(
                        s_t[:, 0:2 * nt, :], ea[:, 0:2 * nt, :], SCH_A, SCH_B,
                        mybir.AluOpType.mult, mybir.AluOpType.add,
                    )
                return (c, s_t, jbs, gi == 0, jbs[-1] == njb - 1)

            for c, gi in items:
                deferred.append(work(c, gi))
                if len(deferred) > DEFER:
                    emit_accums(deferred.pop(0))
            while deferred:
                emit_accums(deferred.pop(0))

    _split_sync_waits(nc)
    return nc


def _split3_e4(v):
    parts = []
    r = np.asarray(v, np.float32)
    for _ in range(3):
        p = r.astype(E4NP).astype(np.float32)
        parts.append(p)
        r = r - p
    return parts


def _build_features(gf, sigma, wv):
    """24 (L_row, R_row) pairs of e4m3 rows s.t. sum_r L_r[j]*R_r[i] =
    -d2[j,i]/(2 sigma^2) + ln(wv) + SCALE_OCT*ln2 to ~1e-2 abs."""
    gf = np.asarray(gf, np.float32)
    n = gf.shape[0]
    g = np.float32(1.0 / (2.0 * sigma * sigma))
    sq = (gf * gf).sum(axis=1)
    a = 2.0 * g * gf
    b = gf
    dh = -g * sq * 0.5
    ch = (-g * sq + np.float32(np.log(wv))
          + np.float32(SCALE_OCT * np.log(2.0))) * 0.5

    A = _split3_e4(a)
    B = _split3_e4(b)
    D = _split3_e4(dh)
    C = _split3_e4(ch)
    ones = np.ones(n, np.float32)
    twos = 2.0 * ones

    Lrows, Rrows = [], []
    for ka, kb in [(0, 0), (0, 1), (1, 0), (1, 1), (0, 2), (2, 0)]:
        for d in range(DG):
            Lrows.append(A[ka][:, d])
            Rrows.append(B[kb][:, d])
    for k in range(3):
        Lrows.append(D[k])
        Rrows.append(twos)
    for k in range(3):
        Lrows.append(twos)
        Rrows.append(C[k])
    assert len(Lrows) == 2 * KI
    L = np.zeros((KI, 2, n), E4NP)
    R = np.zeros((KI, 2, n), E4NP)
    for r in range(2 * KI):
        L[r % KI, r // KI] = Lrows[r].astype(E4NP)
        R[r % KI, r // KI] = Rrows[r].astype(E4NP)
    return L, R


def _pool_operands(gf, sigma, wv, n_cores):
    """GPSIMD-lane operands: AS [128, 4*njb] f32 (a0,a1,a2,d per j), and
    per-core BR (i-side gf rows, f16) / CR (c per i, f32) replicated over
    partitions."""
    gf = np.asarray(gf, np.float32)
    n = gf.shape[0]
    njb = n // 128
    ni = n // n_cores
    g = np.float32(1.0 / (2.0 * sigma * sigma))
    sq = (gf * gf).sum(axis=1)
    a = 2.0 * g * gf                      # [n, 3] j-side
    dterm = -g * sq                       # [n] j-side
    cterm = (-g * sq + np.float32(np.log(wv))
             + np.float32(SCALE_OCT * np.log(2.0)))  # [n] i-side
    AS = np.zeros((128, 4 * njb), np.float32)
    for d in range(3):
        AS[:, d * njb:(d + 1) * njb] = a[:, d].reshape(njb, 128).T
    AS[:, 3 * njb:4 * njb] = dterm.reshape(njb, 128).T
    BRs, CRs = [], []
    for c in range(n_cores):
        rows = slice(c * ni, (c + 1) * ni)
        BRs.append([np.broadcast_to(gf[rows, d].astype(np.float16), (128, ni))
                    .copy() for d in range(3)])
        CRs.append(np.broadcast_to(cterm[rows], (128, ni)).copy())
    return AS, BRs, CRs


def _prepare_inputs(gf0, gf1, node_v_feats, weights, sigmas, n_cores=NCORES):
    weights = np.asarray(weights, np.float32)
    sigmas = np.asarray(sigmas, np.float32)
    V = np.asarray(node_v_feats, np.float32)
    n = V.shape[0]
    ni = n // n_cores
    njb = n // 128

    L0, R0full = _build_features(gf0, float(sigmas[0]), float(weights[0]))
    L1, R1full = _build_features(gf1, float(sigmas[1]), float(weights[1]))
    AS0, BR0s, CR0s = _pool_operands(gf0, float(sigmas[0]), float(weights[0]), n_cores)
    AS1, BR1s, CR1s = _pool_operands(gf1, float(sigmas[1]), float(weights[1]), n_cores)

    vaug = np.concatenate(
        [V.astype(E4NP).astype(np.float32), np.ones((n, 1), np.float32)], axis=1
    ).astype(E4NP)                                    # [n, 129]
    va = np.zeros((128, 2 * njb, 129), E4NP)
    blocks = vaug.reshape(njb, 128, 129)
    for m in range(2):
        va[:, m::2, :] = blocks.transpose(1, 0, 2)

    in_maps = []
    for c in range(n_cores):
        rows = slice(c * ni, (c + 1) * ni)
        vr = np.ascontiguousarray(
            V[rows].reshape(ni // 128, 128, DV).transpose(1, 0, 2).reshape(128, ni)
        )
        in_maps.append({
            "L0": np.ascontiguousarray(L0),
            "L1": np.ascontiguousarray(L1),
            "R0": np.ascontiguousarray(R0full[:, :, rows]),
            "R1": np.ascontiguousarray(R1full[:, :, rows]),
            "VA": va,
            "VR": vr,
            "AS0": AS0, "AS1": AS1,
            "BR00": BR0s[c][0], "BR01": BR0s[c][1], "BR02": BR0s[c][2],
            "BR10": BR1s[c][0], "BR11": BR1s[c][1], "BR12": BR1s[c][2],
            "CR0": CR0s[c], "CR1": CR1s[c],
        })
    return in_maps


_NC_CACHE = {}


def _get_nc(n_j=N, n_i=NI):
    key = (n_j, n_i)
    if key not in _NC_CACHE:
        _NC_CACHE[key] = build_nc(n_j, n_i)
    return _NC_CACHE[key]


_EXEC_CACHE = {}


def _get_executor(nc, n_cores):
    key = (id(nc), n_cores)
    if key in _EXEC_CACHE:
        return _EXEC_CACHE[key]
    import jax
    from jax.experimental.shard_map import shard_map
    from jax.sharding import Mesh, PartitionSpec
    from concourse.bass2jax import (
        install_neuronx_cc_hook,
        _bass_exec_p,
        partition_id_tensor,
    )

    install_neuronx_cc_hook()

    partition_name = nc.partition_id_tensor.name if nc.partition_id_tensor else None
    in_names, out_names, out_avals = [], [], []
    for alloc in nc.m.functions[0].allocations:
        if not isinstance(alloc, mybir.MemoryLocationSet):
            continue
        name = alloc.memorylocations[0].name
        if alloc.kind == "ExternalInput":
            if name != partition_name:
                in_names.append(name)
        elif alloc.kind == "ExternalOutput":
            out_names.append(name)
            out_avals.append(
                jax.core.ShapedArray(tuple(alloc.tensor_shape), mybir.dt.np(alloc.dtype))
            )
    n_params = len(in_names)
    all_names = list(in_names) + list(out_names)
    if partition_name is not None:
        all_names.append(partition_name)

    def _body(*args):
        operands = list(args)
        if partition_name is not None:
            operands.append(partition_id_tensor())
        outs = _bass_exec_p.bind(
            *operands,
            out_avals=tuple(out_avals),
            in_names=tuple(all_names),
            out_names=tuple(out_names),
            lowering_input_output_aliases=(),
            sim_require_finite=True,
            sim_require_nnan=True,
            nc=nc,
        )
        return tuple(outs)

    devices = jax.devices()[:n_cores]
    mesh = Mesh(np.asarray(devices), ("core",))
    n_outs = len(out_names)
    replicated = frozenset(["L0", "L1", "VA", "AS0", "AS1"])
    in_specs = tuple(
        PartitionSpec() if name in replicated else PartitionSpec("core")
        for name in in_names
    ) + (PartitionSpec("core"),) * n_outs
    sharded = jax.jit(
        shard_map(
            _body,
            mesh=mesh,
            in_specs=in_specs,
            out_specs=(PartitionSpec("core"),) * n_outs,
            check_rep=False,
        ),
        donate_argnums=tuple(range(n_params, n_params + n_outs)),
        keep_unused=True,
    )
    entry = (sharded, in_names, out_names, out_avals, replicated)
    _EXEC_CACHE[key] = entry
    return entry


def _run(nc, in_maps, n_cores):
    sharded, in_names, out_names, out_avals, replicated = _get_executor(nc, n_cores)
    concat_in = [
        in_maps[0][name] if name in replicated
        else np.concatenate([in_maps[c][name] for c in range(n_cores)], axis=0)
        for name in in_names
    ]
    concat_zeros = [
        np.zeros((n_cores * a.shape[0], *a.shape[1:]), a.dtype) for a in out_avals
    ]
    out_arrs = sharded(*concat_in, *concat_zeros)
    return [
        {
            name: np.asarray(out_arrs[i]).reshape(n_cores, *out_avals[i].shape)[c]
            for i, name in enumerate(out_names)
        }
        for c in range(n_cores)
    ]


def kernel(gf0, gf1, node_v_feats, weights, sigmas):
    import jax

    in_maps = _prepare_inputs(gf0, gf1, node_v_feats, weights, sigmas)
    nc = _get_nc()
    last_exc = None
    for attempt in range(3):
        try:
            results = _run(nc, in_maps, NCORES)
            jax.effects_barrier()
            out = np.concatenate([results[c]["out"] for c in range(NCORES)], axis=0)
            return np.ascontiguousarray(out.astype(np.float32))
        except Exception as e:
            last_exc = e
            _EXEC_CACHE.clear()
            try:
                jax.clear_caches()
            except Exception:
                pass
            try:
                jax._src.xla_bridge.backends.cache_clear()  # type: ignore[attr-defined]
            except Exception:
                pass
            import time as _time
            _time.sleep(5 * (attempt + 1))
    raise last_exc


# revision 5
# speedup vs baseline: 1.0351x; 1.0103x over previous
"""Trainium2 Bass kernel v2 for nn_GAttn_67147518705771.

Computes: score = w0*RBF(gf0, s0) + w1*RBF(gf1, s1)  (N x N)
          attn  = score / (rowsum(score) + 0.01)
          out   = attn @ V + V

Row-parallel over 8 cores; core c owns output rows [c*1024, (c+1)*1024).

v2 strategy (vs the f16 baseline):
  * All matmuls are fp8e4 + DoubleRow (0.5 cyc/col, 256-row contraction):
    - E matmul per modality: 24 e4m3 feature rows as [12, 2, *] pairs
      produce E[j,i]*1 with ln(w) + 5*ln2 folded in (S scaled x32 so the
      e4m3 subnormal band sits harmlessly low; epilogue uses EPS*32).
    - Accumulation pairs the TWO MODALITY S tiles in one DoubleRow matmul
      against a byte-duplicated VA block [128, 2, 129] ([V|1] e4m3), so
      S0@VA + S1@VA costs 64.5 PE cycles per (jb, i128) block.
  * exp runs on ACT (native Exp -> fp8 out) and DVE (Schraudolph affine ->
    u8, bitcast e4m3). GPSIMD has no PSUM port so it only issues DMAs.
  * PSUM: 3 x [128,4,256] E tiles (2 banks each) + 2 x U accumulator banks.
"""

import numpy as np
import ml_dtypes

import concourse.bass as bass
import concourse.tile as tile
import concourse.mybir as mybir

E4NP = ml_dtypes.float8_e4m3
EPS = 0.01
N = 8192
DG = 3
DV = 128
NCORES = 8
NI = N // NCORES
KI = 12            # feature pair-rows per modality (24 rows via DoubleRow)
CHUNK = 256
NJB = N // 128
SCALE_OCT = 5      # S scaled by 2^5; epilogue divides by (rowsum + EPS*32)
EPS_S = EPS * (1 << SCALE_OCT)
SCH_A = float(np.float32(8.0 * 1.4426950408889634))
SCH_B = float(np.float32(55.537))
DEFER = 3          # groups between exp emission and its accum matmuls

# per-group exp cost estimates (ns) for greedy ACT/DVE balancing
COST = {
    ("act", 1): 612.0, ("act", 2): 1038.0,
    ("dve", 1): 658.0, ("dve", 2): 1192.0,
}


def _split_sync_waits(nc, maxw=1):
    """Walrus rejects instructions with >1 sync waits. Hoist extras onto
    single-wait InstNoOp carriers (same engine). The kernel-tail SP drain's
    waits are spread round-robin across all engines (barrier follows)."""
    n_split = n_carriers = 0
    eng_rr = [
        mybir.EngineType.SP,
        mybir.EngineType.Activation,
        mybir.EngineType.DVE,
        mybir.EngineType.PE,
        mybir.EngineType.Pool,
    ]
    for f in nc.m.functions:
        for bb in f.blocks:
            insts = list(bb.instructions)
            out, changed = [], False
            for inst in insts:
                si = inst.sync_info
                waits = list(si.on_wait) if si and si.on_wait else []
                if len(waits) > maxw:
                    n_split += 1
                    changed = True
                    is_tail_drain = (
                        isinstance(inst, mybir.InstDrain)
                        and inst.engine == mybir.EngineType.SP
                        and len(waits) > 2
                    )
                    for k, w in enumerate(waits[:-maxw]):
                        nop = mybir.InstNoOp(name=f"waitnop-{n_carriers}", ins=[], outs=[])
                        n_carriers += 1
                        nop.engine = eng_rr[k % len(eng_rr)] if is_tail_drain else inst.engine
                        nop.sync_info = mybir.SyncInfo(on_wait=[w], on_update=[])
                        out.append(nop)
                    inst.sync_info = mybir.SyncInfo(
                        on_wait=waits[-maxw:], on_update=list(si.on_update or [])
                    )
                out.append(inst)
            if changed:
                bb.instructions = out
    return n_split, n_carriers


def build_nc(n_j=N, n_i=NI):
    f32 = mybir.dt.float32
    fp8 = mybir.dt.float8e4
    u8 = mybir.dt.uint8
    njb = n_j // 128
    nchunks = n_i // CHUNK

    nc = bass.Bass("TRN2", target_bir_lowering=False, debug=False)
    L0 = nc.dram_tensor("L0", [KI, 2, n_j], fp8, kind="ExternalInput").ap()
    L1 = nc.dram_tensor("L1", [KI, 2, n_j], fp8, kind="ExternalInput").ap()
    R0 = nc.dram_tensor("R0", [KI, 2, n_i], fp8, kind="ExternalInput").ap()
    R1 = nc.dram_tensor("R1", [KI, 2, n_i], fp8, kind="ExternalInput").ap()
    # VA[p, 2*jb+m, v] = [V|1] row (jb*128+p), col v — duplicated per pair m.
    VA = nc.dram_tensor("VA", [128, 2 * njb, 129], fp8, kind="ExternalInput").ap()
    VR = nc.dram_tensor("VR", [128, n_i], f32, kind="ExternalInput").ap()
    # Pool-lane operands: j-side scalars (replicated across cores) and i-side
    # replicated rows (per-core) for computing E on GPSIMD entirely in SBUF.
    AS = [nc.dram_tensor(f"AS{m}", [128, 4 * njb], f32, kind="ExternalInput").ap()
          for m in range(2)]
    BR = [[nc.dram_tensor(f"BR{m}{d}", [128, n_i], mybir.dt.float16,
                          kind="ExternalInput").ap() for d in range(3)]
          for m in range(2)]
    CR = [nc.dram_tensor(f"CR{m}", [128, n_i], f32, kind="ExternalInput").ap()
          for m in range(2)]
    OUT = nc.dram_tensor("out", [n_i, DV], f32, kind="ExternalOutput").ap()

    # per-chunk groups: first chunk leads with a single j-block so the first
    # exp starts early; last chunk ends with singles for a short tail drain.
    # POOL_JBS are computed by the GPSIMD lane instead (late in chunk 0 so its
    # operand DMAs have landed).
    POOL_JBS = {0: [57], 1: [9, 41], 2: [9, 41], 3: [9, 41]}
    POOL_JBS = {c: POOL_JBS.get(c, []) for c in range(nchunks)}

    def mk_groups(c, first_single, last_singles):
        rem = [j for j in range(njb) if j not in POOL_JBS[c]]
        gs = []
        if first_single:
            gs.append([rem.pop(0)])
        tail = [rem.pop(), rem.pop()] if last_singles else []
        while len(rem) >= 2:
            gs.append([rem.pop(0), rem.pop(0)])
        if rem:
            gs.append([rem.pop(0)])
        for j in sorted(tail):
            gs.append([j])
        return gs

    with tile.TileContext(nc) as tc:
        with (
            tc.tile_pool(name="resident", bufs=1) as rpool,
            tc.tile_pool(name="eapool", bufs=3, space="PSUM") as eapool,
            tc.tile_pool(name="upool", bufs=2, space="PSUM") as upool,
            tc.tile_pool(name="spool", bufs=6) as spool,
            tc.tile_pool(name="opool", bufs=8) as opool,
            tc.tile_pool(name="scalars", bufs=4) as scpool,
        ):
            # --- resident inputs, spread across DMA-issue rails ---
            l0_t = rpool.tile([KI, 2, n_j], fp8)
            l1_t = rpool.tile([KI, 2, n_j], fp8)
            r0_t = rpool.tile([KI, 2, n_i], fp8)
            r1_t = rpool.tile([KI, 2, n_i], fp8)
            va_t = rpool.tile([128, 2 * njb, 129], fp8)
            vr_t = rpool.tile([128, n_i], f32)
            as_t = [rpool.tile([128, 4 * njb], f32, name=f"as{m}")
                    for m in range(2)]
            br_t = [[rpool.tile([128, n_i], mybir.dt.float16, name=f"br{m}{d}")
                     for d in range(3)] for m in range(2)]
            cr_t = [rpool.tile([128, n_i], f32, name=f"cr{m}") for m in range(2)]

            # Modality-0 inputs ride sync/HWDGE, modality-1 rides the Pool
            # SWDGE path so their configs run in parallel (HWDGE configs
            # serialize at ~625ns on one shared device); ACT issues no DMAs so
            # its sequencer reaches the first exp immediately.
            PIECE = 8  # j-blocks per L piece
            nc.sync.dma_start(r0_t[:], R0)
            nc.sync.dma_start(l0_t[:, :, 0:PIECE * 128], L0[:, :, 0:PIECE * 128])
            nc.gpsimd.dma_start(r1_t[:], R1)
            nc.gpsimd.dma_start(l1_t[:, :, 0:PIECE * 128], L1[:, :, 0:PIECE * 128])
            nc.gpsimd.dma_start(va_t[:, 0:2 * PIECE, :], VA[:, 0:2 * PIECE, :])
            for p in range(PIECE, njb, PIECE):
                nc.sync.dma_start(
                    l0_t[:, :, p * 128:(p + PIECE) * 128],
                    L0[:, :, p * 128:(p + PIECE) * 128],
                )
                nc.sync.dma_start(
                    l1_t[:, :, p * 128:(p + PIECE) * 128],
                    L1[:, :, p * 128:(p + PIECE) * 128],
                )
                nc.gpsimd.dma_start(
                    va_t[:, 2 * p:2 * (p + PIECE), :], VA[:, 2 * p:2 * (p + PIECE), :]
                )
            for m in range(2):
                nc.sync.dma_start(as_t[m][:], AS[m])
                nc.sync.dma_start(cr_t[m][:], CR[m])
                for d in range(3):
                    nc.sync.dma_start(br_t[m][d][:], BR[m][d])
            nc.sync.dma_start(vr_t[:], VR)

            # ACT exp-table preload + PE p-state warm-up during input DMA.
            dummy = scpool.tile([128, 1], f32, tag="dummy")
            nc.vector.memset(dummy[:], 0.0)
            dummy2 = scpool.tile([128, 1], f32, tag="dummy2")
            nc.scalar.activation(dummy2[:], dummy[:], mybir.ActivationFunctionType.Exp)
            dmm = scpool.tile([1, 256], mybir.dt.bfloat16, tag="dmm")
            nc.vector.memset(dmm[:], 0.0)
            e_warm = eapool.tile([128, 4, 256], f32, tag="ea", name="e_warm")
            for _ in range(14):
                nc.tensor.matmul(
                    e_warm[:, 0, :], lhsT=dmm[:, 0:128], rhs=dmm[:, 0:256],
                    start=True, stop=True,
                )

            # --- global stream of (chunk, group) items with greedy ACT/DVE
            # balance; accum matmuls deferred DEFER groups so the PE never
            # stalls on an in-flight exp, including across chunk boundaries.
            chunk_groups = [mk_groups(0, True, False)] + [
                mk_groups(c, False, False) for c in range(1, nchunks - 1)
            ] + [mk_groups(nchunks - 1, False, True)]
            items = [(c, gi) for c in range(nchunks)
                     for gi in range(len(chunk_groups[c]))]
            t_eng = {"act": 0.0, "dve": 0.0}
            u_tiles = {}
            deferred = []
            pool_s = {}   # (c, jb) -> (s0, s1) u8 tiles

            def emit_pool_chain(c, jb):
                c0 = c * CHUNK
                s01 = []
                for m in range(2):
                    e = opool.tile([128, CHUNK], f32, tag="pe",
                                   name=f"pe_{c}_{jb}_{m}")
                    t = opool.tile([128, CHUNK], f32, tag="pt",
                                   name=f"pt_{c}_{jb}_{m}")
                    nc.gpsimd.tensor_scalar(
                        e[:], br_t[m][0][:, c0:c0 + CHUNK],
                        as_t[m][:, 0 * njb + jb:0 * njb + jb + 1],
                        as_t[m][:, 3 * njb + jb:3 * njb + jb + 1],
                        mybir.AluOpType.mult, mybir.AluOpType.add)
                    for d in (1, 2):
                        nc.gpsimd.tensor_scalar(
                            t[:], br_t[m][d][:, c0:c0 + CHUNK],
                            as_t[m][:, d * njb + jb:d * njb + jb + 1], None,
                            mybir.AluOpType.mult)
                        nc.gpsimd.tensor_add(e[:], e[:], t[:])
                    nc.gpsimd.tensor_add(e[:], e[:], cr_t[m][:, c0:c0 + CHUNK])
                    s_m = spool.tile([128, CHUNK], u8, tag=f"ps{m}",
                                     name=f"ps_{c}_{jb}_{m}")
                    nc.gpsimd.tensor_scalar(
                        s_m[:], e[:], SCH_A, SCH_B,
                        mybir.AluOpType.mult, mybir.AluOpType.add)
                    s01.append(s_m)
                pool_s[(c, jb)] = s01

            def emit_pool_accums(c):
                # plain (non-DoubleRow) fp8 matmuls; PE has ample slack
                u_t = u_tiles[c]
                for jb in POOL_JBS[c]:
                    s0, s1 = pool_s.pop((c, jb))
                    for isub in range(2):
                        for m, s_m in ((0, s0), (1, s1)):
                            nc.tensor.matmul(
                                u_t[:, isub * 256:isub * 256 + 129],
                                lhsT=s_m[:, isub * 128:(isub + 1) * 128].bitcast(
                                    mybir.dt.float8e4),
                                rhs=va_t[:, 2 * jb + m:2 * jb + m + 1, :],
                                start=False, stop=False,
                                skip_group_check=True,
                            )

            def emit_accums(item):
                c, s_t, jbs, first, last = item
                if last:
                    emit_pool_accums(c)
                u_t = u_tiles[c]
                for t, jb in enumerate(jbs):
                    for isub in range(2):
                        # DR psum writes need >=1KB-aligned offsets: slots at
                        # f32 cols 0 and 256 of a full bank.
                        nc.tensor.matmul(
                            u_t[:, isub * 256:isub * 256 + 129],
                            lhsT=s_t[:, 2 * t:2 * t + 2,
                                     isub * 128:(isub + 1) * 128].bitcast(
                                         mybir.dt.float8e4),
                            rhs=va_t[:, 2 * jb:2 * jb + 2, :],
                            start=(first and t == 0 and isub == 0),
                            stop=(last and t == len(jbs) - 1 and isub == 1),
                            skip_group_check=True,
                            perf_mode=mybir.MatmulPerfMode.DoubleRow,
                        )
                if last:
                    emit_epilogue(c)

            def emit_epilogue(c):
                # out rows = U/(rowsum+EPS_S) + V residual; residual add and
                # store run on Pool/SP rails, PSUM-side ops on DVE.
                u_t = u_tiles.pop(c)
                for isub in range(2):
                    g = c * 2 + isub
                    ub = u_t[:, isub * 256:isub * 256 + 129]
                    rt = scpool.tile([128, 1], f32, tag="rt", name=f"rt_{g}")
                    nc.vector.tensor_scalar_add(rt[:], ub[:, 128:129], EPS_S)
                    ri = scpool.tile([128, 1], f32, tag="ri", name=f"ri_{g}")
                    nc.vector.reciprocal(ri[:], rt[:])
                    ot = opool.tile([128, DV], f32, tag="ot", name=f"ot_{g}")
                    nc.vector.tensor_scalar_mul(ot[:], ub[:, 0:DV], ri[:])
                    nc.gpsimd.tensor_add(ot[:], ot[:],
                                         vr_t[:, g * 128:(g + 1) * 128])
                    out_eng = (nc.sync if (isub == 0 or c == nchunks - 1)
                               else nc.gpsimd)
                    out_eng.dma_start(OUT[g * 128:(g + 1) * 128, :], ot[:])

            def work(c, gi):
                jbs = chunk_groups[c][gi]
                nt = len(jbs)
                c0 = c * CHUNK
                if gi == 0:
                    u_tiles[c] = upool.tile([128, 512], f32, tag="u",
                                            name=f"u_{c}")
                    if c == 0:
                        for jb in POOL_JBS[0]:
                            emit_pool_chain(0, jb)
                    if c + 1 < nchunks:
                        # next chunk's Pool chains start a whole chunk early
                        # so they always beat their accum due-time
                        for jb in POOL_JBS[c + 1]:
                            emit_pool_chain(c + 1, jb)
                ea = eapool.tile([128, 4, 256], f32, tag="ea",
                                 name=f"ea_{c}_{jbs[0]}")
                for t, jb in enumerate(jbs):
                    for m, (lt, rt) in enumerate(((l0_t, r0_t), (l1_t, r1_t))):
                        nc.tensor.matmul(
                            ea[:, 2 * t + m, :],
                            lhsT=lt[:, :, jb * 128:(jb + 1) * 128],
                            rhs=rt[:, :, c0:c0 + CHUNK],
                            start=True, stop=True,
                            perf_mode=mybir.MatmulPerfMode.DoubleRow,
                        )
                s_t = spool.tile([128, 4, 256], u8, tag="s",
                                 name=f"s_{c}_{jbs[0]}")
                is_final = (c == nchunks - 1 and gi >= len(chunk_groups[c]) - 1)
                eng = ("act" if is_final else
                       min(("act", "dve"), key=lambda e: t_eng[e] + COST[(e, nt)]))
                t_eng[eng] += COST[(eng, nt)]
                if jbs[-1] == njb - 1:
                    t_eng["dve"] += 700.0  # epilogue PSUM-side ops land on DVE
                if eng == "act":
                    nc.scalar.activation(
                        s_t[:, 0:2 * nt, :].bitcast(mybir.dt.float8e4),
                        ea[:, 0:2 * nt, :],
                        mybir.ActivationFunctionType.Exp,
                    )
                else:
                    nc.vector.tensor_scalar(
                        s_t[:, 0:2 * nt, :], ea[:, 0:2 * nt, :], SCH_A, SCH_B,
                        mybir.AluOpType.mult, mybir.AluOpType.add,
                    )
                return (c, s_t, jbs, gi == 0, jbs[-1] == njb - 1)

            for c, gi in items:
                deferred.append(work(c, gi))
                if len(deferred) > DEFER:
                    emit_accums(deferred.pop(0))
            while deferred:
                emit_accums(deferred.pop(0))

    _split_sync_waits(nc)
    return nc


def _split3_e4(v):
    parts = []
    r = np.asarray(v, np.float32)
    for _ in range(3):
        p = r.astype(E4NP).astype(np.float32)
        parts.append(p)
        r = r - p
    return parts


def _build_features(gf, sigma, wv):
    """24 (L_row, R_row) pairs of e4m3 rows s.t. sum_r L_r[j]*R_r[i] =
    -d2[j,i]/(2 sigma^2) + ln(wv) + SCALE_OCT*ln2 to ~1e-2 abs."""
    gf = np.asarray(gf, np.float32)
    n = gf.shape[0]
    g = np.float32(1.0 / (2.0 * sigma * sigma))
    sq = (gf * gf).sum(axis=1)
    a = 2.0 * g * gf
    b = gf
    dh = -g * sq * 0.5
    ch = (-g * sq + np.float32(np.log(wv))
          + np.float32(SCALE_OCT * np.log(2.0))) * 0.5

    A = _split3_e4(a)
    B = _split3_e4(b)
    D = _split3_e4(dh)
    C = _split3_e4(ch)
    ones = np.ones(n, np.float32)
    twos = 2.0 * ones

    Lrows, Rrows = [], []
    for ka, kb in [(0, 0), (0, 1), (1, 0), (1, 1), (0, 2), (2, 0)]:
        for d in range(DG):
            Lrows.append(A[ka][:, d])
            Rrows.append(B[kb][:, d])
    for k in range(3):
        Lrows.append(D[k])
        Rrows.append(twos)
    for k in range(3):
        Lrows.append(twos)
        Rrows.append(C[k])
    assert len(Lrows) == 2 * KI
    L = np.zeros((KI, 2, n), E4NP)
    R = np.zeros((KI, 2, n), E4NP)
    for r in range(2 * KI):
        L[r % KI, r // KI] = Lrows[r].astype(E4NP)
        R[r % KI, r // KI] = Rrows[r].astype(E4NP)
    return L, R


def _pool_operands(gf, sigma, wv, n_cores):
    """GPSIMD-lane operands: AS [128, 4*njb] f32 (a0,a1,a2,d per j), and
    per-core BR (i-side gf rows, f16) / CR (c per i, f32) replicated over
    partitions."""
    gf = np.asarray(gf, np.float32)
    n = gf.shape[0]
    njb = n // 128
    ni = n // n_cores
    g = np.float32(1.0 / (2.0 * sigma * sigma))
    sq = (gf * gf).sum(axis=1)
    a = 2.0 * g * gf                      # [n, 3] j-side
    dterm = -g * sq                       # [n] j-side
    cterm = (-g * sq + np.float32(np.log(wv))
             + np.float32(SCALE_OCT * np.log(2.0)))  # [n] i-side
    AS = np.zeros((128, 4 * njb), np.float32)
    for d in range(3):
        AS[:, d * njb:(d + 1) * njb] = a[:, d].reshape(njb, 128).T
    AS[:, 3 * njb:4 * njb] = dterm.reshape(njb, 128).T
    BRs, CRs = [], []
    for c in range(n_cores):
        rows = slice(c * ni, (c + 1) * ni)
        BRs.append([np.broadcast_to(gf[rows, d].astype(np.float16), (128, ni))
                    .copy() for d in range(3)])
        CRs.append(np.broadcast_to(cterm[rows], (128, ni)).copy())
    return AS, BRs, CRs


def _prepare_inputs(gf0, gf1, node_v_feats, weights, sigmas, n_cores=NCORES):
    weights = np.asarray(weights, np.float32)
    sigmas = np.asarray(sigmas, np.float32)
    V = np.asarray(node_v_feats, np.float32)
    n = V.shape[0]
    ni = n // n_cores
    njb = n // 128

    L0, R0full = _build_features(gf0, float(sigmas[0]), float(weights[0]))
    L1, R1full = _build_features(gf1, float(sigmas[1]), float(weights[1]))
    AS0, BR0s, CR0s = _pool_operands(gf0, float(sigmas[0]), float(weights[0]), n_cores)
    AS1, BR1s, CR1s = _pool_operands(gf1, float(sigmas[1]), float(weights[1]), n_cores)

    vaug = np.concatenate(
        [V.astype(E4NP).astype(np.float32), np.ones((n, 1), np.float32)], axis=1
    ).astype(E4NP)                                    # [n, 129]
    va = np.zeros((128, 2 * njb, 129), E4NP)
    blocks = vaug.reshape(njb, 128, 129)
    for m in range(2):
        va[:, m::2, :] = blocks.transpose(1, 0, 2)

    in_maps = []
    for c in range(n_cores):
        rows = slice(c * ni, (c + 1) * ni)
        vr = np.ascontiguousarray(
            V[rows].reshape(ni // 128, 128, DV).transpose(1, 0, 2).reshape(128, ni)
        )
        in_maps.append({
            "L0": np.ascontiguousarray(L0),
            "L1": np.ascontiguousarray(L1),
            "R0": np.ascontiguousarray(R0full[:, :, rows]),
            "R1": np.ascontiguousarray(R1full[:, :, rows]),
            "VA": va,
            "VR": vr,
            "AS0": AS0, "AS1": AS1,
            "BR00": BR0s[c][0], "BR01": BR0s[c][1], "BR02": BR0s[c][2],
            "BR10": BR1s[c][0], "BR11": BR1s[c][1], "BR12": BR1s[c][2],
            "CR0": CR0s[c], "CR1": CR1s[c],
        })
    return in_maps


_NC_CACHE = {}


def _get_nc(n_j=N, n_i=NI):
    key = (n_j, n_i)
    if key not in _NC_CACHE:
        _NC_CACHE[key] = build_nc(n_j, n_i)
    return _NC_CACHE[key]


_EXEC_CACHE = {}


def _get_executor(nc, n_cores):
    key = (id(nc), n_cores)
    if key in _EXEC_CACHE:
        return _EXEC_CACHE[key]
    import jax
    from jax.experimental.shard_map import shard_map
    from jax.sharding import Mesh, PartitionSpec
    from concourse.bass2jax import (
        install_neuronx_cc_hook,
        _bass_exec_p,
        partition_id_tensor,
    )

    install_neuronx_cc_hook()

    partition_name = nc.partition_id_tensor.name if nc.partition_id_tensor else None
    in_names, out_names, out_avals = [], [], []
    for alloc in nc.m.functions[0].allocations:
        if not isinstance(alloc, mybir.MemoryLocationSet):
            continue
        name = alloc.memorylocations[0].name
        if alloc.kind == "ExternalInput":
            if name != partition_name:
                in_names.append(name)
        elif alloc.kind == "ExternalOutput":
            out_names.append(name)
            out_avals.append(
                jax.core.ShapedArray(tuple(alloc.tensor_shape), mybir.dt.np(alloc.dtype))
            )
    n_params = len(in_names)
    all_names = list(in_names) + list(out_names)
    if partition_name is not None:
        all_names.append(partition_name)

    def _body(*args):
        operands = list(args)
        if partition_name is not None:
            operands.append(partition_id_tensor())
        outs = _bass_exec_p.bind(
            *operands,
            out_avals=tuple(out_avals),
            in_names=tuple(all_names),
            out_names=tuple(out_names),
            lowering_input_output_aliases=(),
            sim_require_finite=True,
            sim_require_nnan=True,
            nc=nc,
        )
        return tuple(outs)

    devices = jax.devices()[:n_cores]
    mesh = Mesh(np.asarray(devices), ("core",))
    n_outs = len(out_names)
    replicated = frozenset(["L0", "L1", "VA", "AS0", "AS1"])
    in_specs = tuple(
        PartitionSpec() if name in replicated else PartitionSpec("core")
        for name in in_names
    ) + (PartitionSpec("core"),) * n_outs
    sharded = jax.jit(
        shard_map(
            _body,
            mesh=mesh,
            in_specs=in_specs,
            out_specs=(PartitionSpec("core"),) * n_outs,
            check_rep=False,
        ),
        donate_argnums=tuple(range(n_params, n_params + n_outs)),
        keep_unused=True,
    )
    entry = (sharded, in_names, out_names, out_avals, replicated)
    _EXEC_CACHE[key] = entry
    return entry


def _run(nc, in_maps, n_cores):
    sharded, in_names, out_names, out_avals, replicated = _get_executor(nc, n_cores)
    concat_in = [
        in_maps[0][name] if name in replicated
        else np.concatenate([in_maps[c][name] for c in range(n_cores)], axis=0)
        for name in in_names
    ]
    concat_zeros = [
        np.zeros((n_cores * a.shape[0], *a.shape[1:]), a.dtype) for a in out_avals
    ]
    out_arrs = sharded(*concat_in, *concat_zeros)
    return [
        {
            name: np.asarray(out_arrs[i]).reshape(n_cores, *out_avals[i].shape)[c]
            for i, name in enumerate(out_names)
        }
        for c in range(n_cores)
    ]


def kernel(gf0, gf1, node_v_feats, weights, sigmas):
    import jax

    in_maps = _prepare_inputs(gf0, gf1, node_v_feats, weights, sigmas)
    nc = _get_nc()
    last_exc = None
    for attempt in range(3):
        try:
            results = _run(nc, in_maps, NCORES)
            jax.effects_barrier()
            out = np.concatenate([results[c]["out"] for c in range(NCORES)], axis=0)
            return np.ascontiguousarray(out.astype(np.float32))
        except Exception as e:
            last_exc = e
            _EXEC_CACHE.clear()
            try:
                jax.clear_caches()
            except Exception:
                pass
            try:
                jax._src.xla_bridge.backends.cache_clear()  # type: ignore[attr-defined]
            except Exception:
                pass
            import time as _time
            _time.sleep(5 * (attempt + 1))
    raise last_exc


# revision 6
# speedup vs baseline: 1.0357x; 1.0006x over previous
"""Trainium2 Bass kernel v2 for nn_GAttn_67147518705771.

Computes: score = w0*RBF(gf0, s0) + w1*RBF(gf1, s1)  (N x N)
          attn  = score / (rowsum(score) + 0.01)
          out   = attn @ V + V

Row-parallel over 8 cores; core c owns output rows [c*1024, (c+1)*1024).

v2 strategy (vs the f16 baseline):
  * All matmuls are fp8e4 + DoubleRow (0.5 cyc/col, 256-row contraction):
    - E matmul per modality: 24 e4m3 feature rows as [12, 2, *] pairs
      produce E[j,i]*1 with ln(w) + 5*ln2 folded in (S scaled x32 so the
      e4m3 subnormal band sits harmlessly low; epilogue uses EPS*32).
    - Accumulation pairs the TWO MODALITY S tiles in one DoubleRow matmul
      against a byte-duplicated VA block [128, 2, 129] ([V|1] e4m3), so
      S0@VA + S1@VA costs 64.5 PE cycles per (jb, i128) block.
  * exp runs on ACT (native Exp -> fp8 out) and DVE (Schraudolph affine ->
    u8, bitcast e4m3). GPSIMD has no PSUM port so it only issues DMAs.
  * PSUM: 3 x [128,4,256] E tiles (2 banks each) + 2 x U accumulator banks.
"""

import numpy as np
import ml_dtypes

import concourse.bass as bass
import concourse.tile as tile
import concourse.mybir as mybir

E4NP = ml_dtypes.float8_e4m3
EPS = 0.01
N = 8192
DG = 3
DV = 128
NCORES = 8
NI = N // NCORES
KI = 12            # feature pair-rows per modality (24 rows via DoubleRow)
CHUNK = 256
NJB = N // 128
SCALE_OCT = 5      # S scaled by 2^5; epilogue divides by (rowsum + EPS*32)
EPS_S = EPS * (1 << SCALE_OCT)
SCH_A = float(np.float32(8.0 * 1.4426950408889634))
SCH_B = float(np.float32(55.537))
DEFER = 3          # groups between exp emission and its accum matmuls

# per-group exp cost estimates (ns) for greedy ACT/DVE balancing
COST = {
    ("act", 1): 612.0, ("act", 2): 1038.0,
    ("dve", 1): 658.0, ("dve", 2): 1192.0,
}


def _split_sync_waits(nc, maxw=1):
    """Walrus rejects instructions with >1 sync waits. Hoist extras onto
    single-wait InstNoOp carriers (same engine). The kernel-tail SP drain's
    waits are spread round-robin across all engines (barrier follows)."""
    n_split = n_carriers = 0
    eng_rr = [
        mybir.EngineType.SP,
        mybir.EngineType.Activation,
        mybir.EngineType.DVE,
        mybir.EngineType.PE,
        mybir.EngineType.Pool,
    ]
    for f in nc.m.functions:
        for bb in f.blocks:
            insts = list(bb.instructions)
            out, changed = [], False
            for inst in insts:
                si = inst.sync_info
                waits = list(si.on_wait) if si and si.on_wait else []
                if len(waits) > maxw:
                    n_split += 1
                    changed = True
                    is_tail_drain = (
                        isinstance(inst, mybir.InstDrain)
                        and inst.engine == mybir.EngineType.SP
                        and len(waits) > 2
                    )
                    for k, w in enumerate(waits[:-maxw]):
                        nop = mybir.InstNoOp(name=f"waitnop-{n_carriers}", ins=[], outs=[])
                        n_carriers += 1
                        nop.engine = eng_rr[k % len(eng_rr)] if is_tail_drain else inst.engine
                        nop.sync_info = mybir.SyncInfo(on_wait=[w], on_update=[])
                        out.append(nop)
                    inst.sync_info = mybir.SyncInfo(
                        on_wait=waits[-maxw:], on_update=list(si.on_update or [])
                    )
                out.append(inst)
            if changed:
                bb.instructions = out
    return n_split, n_carriers


def build_nc(n_j=N, n_i=NI):
    f32 = mybir.dt.float32
    fp8 = mybir.dt.float8e4
    u8 = mybir.dt.uint8
    njb = n_j // 128
    nchunks = n_i // CHUNK

    nc = bass.Bass("TRN2", target_bir_lowering=False, debug=False)
    L0 = nc.dram_tensor("L0", [KI, 2, n_j], fp8, kind="ExternalInput").ap()
    L1 = nc.dram_tensor("L1", [KI, 2, n_j], fp8, kind="ExternalInput").ap()
    R0 = nc.dram_tensor("R0", [KI, 2, n_i], fp8, kind="ExternalInput").ap()
    R1 = nc.dram_tensor("R1", [KI, 2, n_i], fp8, kind="ExternalInput").ap()
    # VA[p, 2*jb+m, v] = [V|1] row (jb*128+p), col v — duplicated per pair m.
    VA = nc.dram_tensor("VA", [128, 2 * njb, 129], fp8, kind="ExternalInput").ap()
    VR = nc.dram_tensor("VR", [128, n_i], f32, kind="ExternalInput").ap()
    # Pool-lane operands: j-side scalars (replicated across cores) and i-side
    # replicated rows (per-core) for computing E on GPSIMD entirely in SBUF.
    AS = [nc.dram_tensor(f"AS{m}", [128, 4 * njb], f32, kind="ExternalInput").ap()
          for m in range(2)]
    BR = [[nc.dram_tensor(f"BR{m}{d}", [128, n_i], mybir.dt.float16,
                          kind="ExternalInput").ap() for d in range(3)]
          for m in range(2)]
    CR = [nc.dram_tensor(f"CR{m}", [128, n_i], f32, kind="ExternalInput").ap()
          for m in range(2)]
    OUT = nc.dram_tensor("out", [n_i, DV], f32, kind="ExternalOutput").ap()

    # per-chunk groups: first chunk leads with a single j-block so the first
    # exp starts early; last chunk ends with singles for a short tail drain.
    # POOL_JBS are computed by the GPSIMD lane instead (late in chunk 0 so its
    # operand DMAs have landed).
    POOL_JBS = {0: [], 1: [9, 41], 2: [9, 41], 3: [9, 41]}
    POOL_JBS = {c: POOL_JBS.get(c, []) for c in range(nchunks)}

    def mk_groups(c, first_single, last_singles):
        rem = [j for j in range(njb) if j not in POOL_JBS[c]]
        gs = []
        if first_single:
            gs.append([rem.pop(0)])
        tail = [rem.pop(), rem.pop()] if last_singles else []
        while len(rem) >= 2:
            gs.append([rem.pop(0), rem.pop(0)])
        if rem:
            gs.append([rem.pop(0)])
        for j in sorted(tail):
            gs.append([j])
        return gs

    with tile.TileContext(nc) as tc:
        with (
            tc.tile_pool(name="resident", bufs=1) as rpool,
            tc.tile_pool(name="eapool", bufs=3, space="PSUM") as eapool,
            tc.tile_pool(name="upool", bufs=2, space="PSUM") as upool,
            tc.tile_pool(name="spool", bufs=6) as spool,
            tc.tile_pool(name="opool", bufs=8) as opool,
            tc.tile_pool(name="scalars", bufs=4) as scpool,
        ):
            # --- resident inputs, spread across DMA-issue rails ---
            l0_t = rpool.tile([KI, 2, n_j], fp8)
            l1_t = rpool.tile([KI, 2, n_j], fp8)
            r0_t = rpool.tile([KI, 2, n_i], fp8)
            r1_t = rpool.tile([KI, 2, n_i], fp8)
            va_t = rpool.tile([128, 2 * njb, 129], fp8)
            vr_t = rpool.tile([128, n_i], f32)
            as_t = [rpool.tile([128, 4 * njb], f32, name=f"as{m}")
                    for m in range(2)]
            br_t = [[rpool.tile([128, n_i], mybir.dt.float16, name=f"br{m}{d}")
                     for d in range(3)] for m in range(2)]
            cr_t = [rpool.tile([128, n_i], f32, name=f"cr{m}") for m in range(2)]

            # Modality-0 inputs ride sync/HWDGE, modality-1 rides the Pool
            # SWDGE path so their configs run in parallel (HWDGE configs
            # serialize at ~625ns on one shared device); ACT issues no DMAs so
            # its sequencer reaches the first exp immediately.
            PIECE = 8  # j-blocks per L piece
            nc.sync.dma_start(r0_t[:], R0)
            nc.sync.dma_start(l0_t[:, :, 0:PIECE * 128], L0[:, :, 0:PIECE * 128])
            nc.gpsimd.dma_start(r1_t[:], R1)
            nc.gpsimd.dma_start(l1_t[:, :, 0:PIECE * 128], L1[:, :, 0:PIECE * 128])
            nc.gpsimd.dma_start(va_t[:, 0:2 * PIECE, :], VA[:, 0:2 * PIECE, :])
            for p in range(PIECE, njb, PIECE):
                nc.sync.dma_start(
                    l0_t[:, :, p * 128:(p + PIECE) * 128],
                    L0[:, :, p * 128:(p + PIECE) * 128],
                )
                nc.sync.dma_start(
                    l1_t[:, :, p * 128:(p + PIECE) * 128],
                    L1[:, :, p * 128:(p + PIECE) * 128],
                )
                nc.gpsimd.dma_start(
                    va_t[:, 2 * p:2 * (p + PIECE), :], VA[:, 2 * p:2 * (p + PIECE), :]
                )
            for m in range(2):
                nc.sync.dma_start(as_t[m][:], AS[m])
                nc.sync.dma_start(cr_t[m][:], CR[m])
                for d in range(3):
                    nc.sync.dma_start(br_t[m][d][:], BR[m][d])
            nc.sync.dma_start(vr_t[:], VR)

            # ACT exp-table preload + PE p-state warm-up during input DMA.
            dummy = scpool.tile([128, 1], f32, tag="dummy")
            nc.vector.memset(dummy[:], 0.0)
            dummy2 = scpool.tile([128, 1], f32, tag="dummy2")
            nc.scalar.activation(dummy2[:], dummy[:], mybir.ActivationFunctionType.Exp)
            dmm = scpool.tile([1, 256], mybir.dt.bfloat16, tag="dmm")
            nc.vector.memset(dmm[:], 0.0)
            e_warm = eapool.tile([128, 4, 256], f32, tag="ea", name="e_warm")
            for _ in range(14):
                nc.tensor.matmul(
                    e_warm[:, 0, :], lhsT=dmm[:, 0:128], rhs=dmm[:, 0:256],
                    start=True, stop=True,
                )

            # --- global stream of (chunk, group) items with greedy ACT/DVE
            # balance; accum matmuls deferred DEFER groups so the PE never
            # stalls on an in-flight exp, including across chunk boundaries.
            chunk_groups = [mk_groups(0, True, False)] + [
                mk_groups(c, False, False) for c in range(1, nchunks - 1)
            ] + [mk_groups(nchunks - 1, False, True)]
            items = [(c, gi) for c in range(nchunks)
                     for gi in range(len(chunk_groups[c]))]
            t_eng = {"act": 0.0, "dve": 0.0}
            u_tiles = {}
            deferred = []
            pool_s = {}   # (c, jb) -> (s0, s1) u8 tiles

            def emit_pool_chain(c, jb):
                c0 = c * CHUNK
                s01 = []
                for m in range(2):
                    e = opool.tile([128, CHUNK], f32, tag="pe",
                                   name=f"pe_{c}_{jb}_{m}")
                    t = opool.tile([128, CHUNK], f32, tag="pt",
                                   name=f"pt_{c}_{jb}_{m}")
                    nc.gpsimd.tensor_scalar(
                        e[:], br_t[m][0][:, c0:c0 + CHUNK],
                        as_t[m][:, 0 * njb + jb:0 * njb + jb + 1],
                        as_t[m][:, 3 * njb + jb:3 * njb + jb + 1],
                        mybir.AluOpType.mult, mybir.AluOpType.add)
                    for d in (1, 2):
                        nc.gpsimd.tensor_scalar(
                            t[:], br_t[m][d][:, c0:c0 + CHUNK],
                            as_t[m][:, d * njb + jb:d * njb + jb + 1], None,
                            mybir.AluOpType.mult)
                        nc.gpsimd.tensor_add(e[:], e[:], t[:])
                    nc.gpsimd.tensor_add(e[:], e[:], cr_t[m][:, c0:c0 + CHUNK])
                    s_m = spool.tile([128, CHUNK], u8, tag=f"ps{m}",
                                     name=f"ps_{c}_{jb}_{m}")
                    nc.gpsimd.tensor_scalar(
                        s_m[:], e[:], SCH_A, SCH_B,
                        mybir.AluOpType.mult, mybir.AluOpType.add)
                    s01.append(s_m)
                pool_s[(c, jb)] = s01

            def emit_pool_accums(c):
                # plain (non-DoubleRow) fp8 matmuls; PE has ample slack
                u_t = u_tiles[c]
                for jb in POOL_JBS[c]:
                    s0, s1 = pool_s.pop((c, jb))
                    for isub in range(2):
                        for m, s_m in ((0, s0), (1, s1)):
                            nc.tensor.matmul(
                                u_t[:, isub * 256:isub * 256 + 129],
                                lhsT=s_m[:, isub * 128:(isub + 1) * 128].bitcast(
                                    mybir.dt.float8e4),
                                rhs=va_t[:, 2 * jb + m:2 * jb + m + 1, :],
                                start=False, stop=False,
                                skip_group_check=True,
                            )

            def emit_accums(item):
                c, s_t, jbs, first, last = item
                if last:
                    emit_pool_accums(c)
                u_t = u_tiles[c]
                for t, jb in enumerate(jbs):
                    for isub in range(2):
                        # DR psum writes need >=1KB-aligned offsets: slots at
                        # f32 cols 0 and 256 of a full bank.
                        nc.tensor.matmul(
                            u_t[:, isub * 256:isub * 256 + 129],
                            lhsT=s_t[:, 2 * t:2 * t + 2,
                                     isub * 128:(isub + 1) * 128].bitcast(
                                         mybir.dt.float8e4),
                            rhs=va_t[:, 2 * jb:2 * jb + 2, :],
                            start=(first and t == 0 and isub == 0),
                            stop=(last and t == len(jbs) - 1 and isub == 1),
                            skip_group_check=True,
                            perf_mode=mybir.MatmulPerfMode.DoubleRow,
                        )
                if last:
                    emit_epilogue(c)

            def emit_epilogue(c):
                # out rows = U/(rowsum+EPS_S) + V residual; residual add and
                # store run on Pool/SP rails, PSUM-side ops on DVE.
                u_t = u_tiles.pop(c)
                for isub in range(2):
                    g = c * 2 + isub
                    ub = u_t[:, isub * 256:isub * 256 + 129]
                    rt = scpool.tile([128, 1], f32, tag="rt", name=f"rt_{g}")
                    nc.vector.tensor_scalar_add(rt[:], ub[:, 128:129], EPS_S)
                    ri = scpool.tile([128, 1], f32, tag="ri", name=f"ri_{g}")
                    nc.vector.reciprocal(ri[:], rt[:])
                    ot = opool.tile([128, DV], f32, tag="ot", name=f"ot_{g}")
                    nc.vector.tensor_scalar_mul(ot[:], ub[:, 0:DV], ri[:])
                    nc.gpsimd.tensor_add(ot[:], ot[:],
                                         vr_t[:, g * 128:(g + 1) * 128])
                    out_eng = (nc.sync if (isub == 0 or c == nchunks - 1)
                               else nc.gpsimd)
                    out_eng.dma_start(OUT[g * 128:(g + 1) * 128, :], ot[:])

            def work(c, gi):
                jbs = chunk_groups[c][gi]
                nt = len(jbs)
                c0 = c * CHUNK
                if gi == 0:
                    u_tiles[c] = upool.tile([128, 512], f32, tag="u",
                                            name=f"u_{c}")
                    if c == 0:
                        for jb in POOL_JBS[0]:
                            emit_pool_chain(0, jb)
                    if c + 1 < nchunks:
                        # next chunk's Pool chains start a whole chunk early
                        # so they always beat their accum due-time
                        for jb in POOL_JBS[c + 1]:
                            emit_pool_chain(c + 1, jb)
                ea = eapool.tile([128, 4, 256], f32, tag="ea",
                                 name=f"ea_{c}_{jbs[0]}")
                for t, jb in enumerate(jbs):
                    for m, (lt, rt) in enumerate(((l0_t, r0_t), (l1_t, r1_t))):
                        nc.tensor.matmul(
                            ea[:, 2 * t + m, :],
                            lhsT=lt[:, :, jb * 128:(jb + 1) * 128],
                            rhs=rt[:, :, c0:c0 + CHUNK],
                            start=True, stop=True,
                            perf_mode=mybir.MatmulPerfMode.DoubleRow,
                        )
                s_t = spool.tile([128, 4, 256], u8, tag="s",
                                 name=f"s_{c}_{jbs[0]}")
                is_final = (c == nchunks - 1 and gi >= len(chunk_groups[c]) - 1)
                eng = ("act" if is_final else
                       min(("act", "dve"), key=lambda e: t_eng[e] + COST[(e, nt)]))
                t_eng[eng] += COST[(eng, nt)]
                if jbs[-1] == njb - 1:
                    t_eng["dve"] += 700.0  # epilogue PSUM-side ops land on DVE
                if eng == "act":
                    nc.scalar.activation(
                        s_t[:, 0:2 * nt, :].bitcast(mybir.dt.float8e4),
                        ea[:, 0:2 * nt, :],
                        mybir.ActivationFunctionType.Exp,
                    )
                else:
                    nc.vector.tensor_scalar(
                        s_t[:, 0:2 * nt, :], ea[:, 0:2 * nt, :], SCH_A, SCH_B,
                        mybir.AluOpType.mult, mybir.AluOpType.add,
                    )
                return (c, s_t, jbs, gi == 0, jbs[-1] == njb - 1)

            for c, gi in items:
                deferred.append(work(c, gi))
                if len(deferred) > DEFER:
                    emit_accums(deferred.pop(0))
            while deferred:
                emit_accums(deferred.pop(0))

    _split_sync_waits(nc)
    return nc


def _split3_e4(v):
    parts = []
    r = np.asarray(v, np.float32)
    for _ in range(3):
        p = r.astype(E4NP).astype(np.float32)
        parts.append(p)
        r = r - p
    return parts


def _build_features(gf, sigma, wv):
    """24 (L_row, R_row) pairs of e4m3 rows s.t. sum_r L_r[j]*R_r[i] =
    -d2[j,i]/(2 sigma^2) + ln(wv) + SCALE_OCT*ln2 to ~1e-2 abs."""
    gf = np.asarray(gf, np.float32)
    n = gf.shape[0]
    g = np.float32(1.0 / (2.0 * sigma * sigma))
    sq = (gf * gf).sum(axis=1)
    a = 2.0 * g * gf
    b = gf
    dh = -g * sq * 0.5
    ch = (-g * sq + np.float32(np.log(wv))
          + np.float32(SCALE_OCT * np.log(2.0))) * 0.5

    A = _split3_e4(a)
    B = _split3_e4(b)
    D = _split3_e4(dh)
    C = _split3_e4(ch)
    ones = np.ones(n, np.float32)
    twos = 2.0 * ones

    Lrows, Rrows = [], []
    for ka, kb in [(0, 0), (0, 1), (1, 0), (1, 1), (0, 2), (2, 0)]:
        for d in range(DG):
            Lrows.append(A[ka][:, d])
            Rrows.append(B[kb][:, d])
    for k in range(3):
        Lrows.append(D[k])
        Rrows.append(twos)
    for k in range(3):
        Lrows.append(twos)
        Rrows.append(C[k])
    assert len(Lrows) == 2 * KI
    L = np.zeros((KI, 2, n), E4NP)
    R = np.zeros((KI, 2, n), E4NP)
    for r in range(2 * KI):
        L[r % KI, r // KI] = Lrows[r].astype(E4NP)
        R[r % KI, r // KI] = Rrows[r].astype(E4NP)
    return L, R


def _pool_operands(gf, sigma, wv, n_cores):
    """GPSIMD-lane operands: AS [128, 4*njb] f32 (a0,a1,a2,d per j), and
    per-core BR (i-side gf rows, f16) / CR (c per i, f32) replicated over
    partitions."""
    gf = np.asarray(gf, np.float32)
    n = gf.shape[0]
    njb = n // 128
    ni = n // n_cores
    g = np.float32(1.0 / (2.0 * sigma * sigma))
    sq = (gf * gf).sum(axis=1)
    a = 2.0 * g * gf                      # [n, 3] j-side
    dterm = -g * sq                       # [n] j-side
    cterm = (-g * sq + np.float32(np.log(wv))
             + np.float32(SCALE_OCT * np.log(2.0)))  # [n] i-side
    AS = np.zeros((128, 4 * njb), np.float32)
    for d in range(3):
        AS[:, d * njb:(d + 1) * njb] = a[:, d].reshape(njb, 128).T
    AS[:, 3 * njb:4 * njb] = dterm.reshape(njb, 128).T
    BRs, CRs = [], []
    for c in range(n_cores):
        rows = slice(c * ni, (c + 1) * ni)
        BRs.append([np.broadcast_to(gf[rows, d].astype(np.float16), (128, ni))
                    .copy() for d in range(3)])
        CRs.append(np.broadcast_to(cterm[rows], (128, ni)).copy())
    return AS, BRs, CRs


def _prepare_inputs(gf0, gf1, node_v_feats, weights, sigmas, n_cores=NCORES):
    weights = np.asarray(weights, np.float32)
    sigmas = np.asarray(sigmas, np.float32)
    V = np.asarray(node_v_feats, np.float32)
    n = V.shape[0]
    ni = n // n_cores
    njb = n // 128

    L0, R0full = _build_features(gf0, float(sigmas[0]), float(weights[0]))
    L1, R1full = _build_features(gf1, float(sigmas[1]), float(weights[1]))
    AS0, BR0s, CR0s = _pool_operands(gf0, float(sigmas[0]), float(weights[0]), n_cores)
    AS1, BR1s, CR1s = _pool_operands(gf1, float(sigmas[1]), float(weights[1]), n_cores)

    vaug = np.concatenate(
        [V.astype(E4NP).astype(np.float32), np.ones((n, 1), np.float32)], axis=1
    ).astype(E4NP)                                    # [n, 129]
    va = np.zeros((128, 2 * njb, 129), E4NP)
    blocks = vaug.reshape(njb, 128, 129)
    for m in range(2):
        va[:, m::2, :] = blocks.transpose(1, 0, 2)

    in_maps = []
    for c in range(n_cores):
        rows = slice(c * ni, (c + 1) * ni)
        vr = np.ascontiguousarray(
            V[rows].reshape(ni // 128, 128, DV).transpose(1, 0, 2).reshape(128, ni)
        )
        in_maps.append({
            "L0": np.ascontiguousarray(L0),
            "L1": np.ascontiguousarray(L1),
            "R0": np.ascontiguousarray(R0full[:, :, rows]),
            "R1": np.ascontiguousarray(R1full[:, :, rows]),
            "VA": va,
            "VR": vr,
            "AS0": AS0, "AS1": AS1,
            "BR00": BR0s[c][0], "BR01": BR0s[c][1], "BR02": BR0s[c][2],
            "BR10": BR1s[c][0], "BR11": BR1s[c][1], "BR12": BR1s[c][2],
            "CR0": CR0s[c], "CR1": CR1s[c],
        })
    return in_maps


_NC_CACHE = {}


def _get_nc(n_j=N, n_i=NI):
    key = (n_j, n_i)
    if key not in _NC_CACHE:
        _NC_CACHE[key] = build_nc(n_j, n_i)
    return _NC_CACHE[key]


_EXEC_CACHE = {}


def _get_executor(nc, n_cores):
    key = (id(nc), n_cores)
    if key in _EXEC_CACHE:
        return _EXEC_CACHE[key]
    import jax
    from jax.experimental.shard_map import shard_map
    from jax.sharding import Mesh, PartitionSpec
    from concourse.bass2jax import (
        install_neuronx_cc_hook,
        _bass_exec_p,
        partition_id_tensor,
    )

    install_neuronx_cc_hook()

    partition_name = nc.partition_id_tensor.name if nc.partition_id_tensor else None
    in_names, out_names, out_avals = [], [], []
    for alloc in nc.m.functions[0].allocations:
        if not isinstance(alloc, mybir.MemoryLocationSet):
            continue
        name = alloc.memorylocations[0].name
        if alloc.kind == "ExternalInput":
            if name != partition_name:
                in_names.append(name)
        elif alloc.kind == "ExternalOutput":
            out_names.append(name)
            out_avals.append(
                jax.core.ShapedArray(tuple(alloc.tensor_shape), mybir.dt.np(alloc.dtype))
            )
    n_params = len(in_names)
    all_names = list(in_names) + list(out_names)
    if partition_name is not None:
        all_names.append(partition_name)

    def _body(*args):
        operands = list(args)
        if partition_name is not None:
            operands.append(partition_id_tensor())
        outs = _bass_exec_p.bind(
            *operands,
            out_avals=tuple(out_avals),
            in_names=tuple(all_names),
            out_names=tuple(out_names),
            lowering_input_output_aliases=(),
            sim_require_finite=True,
            sim_require_nnan=True,
            nc=nc,
        )
        return tuple(outs)

    devices = jax.devices()[:n_cores]
    mesh = Mesh(np.asarray(devices), ("core",))
    n_outs = len(out_names)
    replicated = frozenset(["L0", "L1", "VA", "AS0", "AS1"])
    in_specs = tuple(
        PartitionSpec() if name in replicated else PartitionSpec("core")
        for name in in_names
    ) + (PartitionSpec("core"),) * n_outs
    sharded = jax.jit(
        shard_map(
            _body,
            mesh=mesh,
            in_specs=in_specs,
            out_specs=(PartitionSpec("core"),) * n_outs,
            check_rep=False,
        ),
        donate_argnums=tuple(range(n_params, n_params + n_outs)),
        keep_unused=True,
    )
    entry = (sharded, in_names, out_names, out_avals, replicated)
    _EXEC_CACHE[key] = entry
    return entry


def _run(nc, in_maps, n_cores):
    sharded, in_names, out_names, out_avals, replicated = _get_executor(nc, n_cores)
    concat_in = [
        in_maps[0][name] if name in replicated
        else np.concatenate([in_maps[c][name] for c in range(n_cores)], axis=0)
        for name in in_names
    ]
    concat_zeros = [
        np.zeros((n_cores * a.shape[0], *a.shape[1:]), a.dtype) for a in out_avals
    ]
    out_arrs = sharded(*concat_in, *concat_zeros)
    return [
        {
            name: np.asarray(out_arrs[i]).reshape(n_cores, *out_avals[i].shape)[c]
            for i, name in enumerate(out_names)
        }
        for c in range(n_cores)
    ]


def kernel(gf0, gf1, node_v_feats, weights, sigmas):
    import jax

    in_maps = _prepare_inputs(gf0, gf1, node_v_feats, weights, sigmas)
    nc = _get_nc()
    last_exc = None
    for attempt in range(3):
        try:
            results = _run(nc, in_maps, NCORES)
            jax.effects_barrier()
            out = np.concatenate([results[c]["out"] for c in range(NCORES)], axis=0)
            return np.ascontiguousarray(out.astype(np.float32))
        except Exception as e:
            last_exc = e
            _EXEC_CACHE.clear()
            try:
                jax.clear_caches()
            except Exception:
                pass
            try:
                jax._src.xla_bridge.backends.cache_clear()  # type: ignore[attr-defined]
            except Exception:
                pass
            import time as _time
            _time.sleep(5 * (attempt + 1))
    raise last_exc


# revision 7
# speedup vs baseline: 1.0374x; 1.0016x over previous
"""Trainium2 Bass kernel v2 for nn_GAttn_67147518705771.

Computes: score = w0*RBF(gf0, s0) + w1*RBF(gf1, s1)  (N x N)
          attn  = score / (rowsum(score) + 0.01)
          out   = attn @ V + V

Row-parallel over 8 cores; core c owns output rows [c*1024, (c+1)*1024).

v2 strategy (vs the f16 baseline):
  * All matmuls are fp8e4 + DoubleRow (0.5 cyc/col, 256-row contraction):
    - E matmul per modality: 24 e4m3 feature rows as [12, 2, *] pairs
      produce E[j,i]*1 with ln(w) + 5*ln2 folded in (S scaled x32 so the
      e4m3 subnormal band sits harmlessly low; epilogue uses EPS*32).
    - Accumulation pairs the TWO MODALITY S tiles in one DoubleRow matmul
      against a byte-duplicated VA block [128, 2, 129] ([V|1] e4m3), so
      S0@VA + S1@VA costs 64.5 PE cycles per (jb, i128) block.
  * exp runs on ACT (native Exp -> fp8 out) and DVE (Schraudolph affine ->
    u8, bitcast e4m3). GPSIMD has no PSUM port so it only issues DMAs.
  * PSUM: 3 x [128,4,256] E tiles (2 banks each) + 2 x U accumulator banks.
"""

import numpy as np
import ml_dtypes

import concourse.bass as bass
import concourse.tile as tile
import concourse.mybir as mybir

E4NP = ml_dtypes.float8_e4m3
EPS = 0.01
N = 8192
DG = 3
DV = 128
NCORES = 8
NI = N // NCORES
KI = 12            # feature pair-rows per modality (24 rows via DoubleRow)
CHUNK = 256
NJB = N // 128
SCALE_OCT = 5      # S scaled by 2^5; epilogue divides by (rowsum + EPS*32)
EPS_S = EPS * (1 << SCALE_OCT)
SCH_A = float(np.float32(8.0 * 1.4426950408889634))
SCH_B = float(np.float32(55.537))
DEFER = 3          # groups between exp emission and its accum matmuls

# per-group exp cost estimates (ns) for greedy ACT/DVE balancing
COST = {
    ("act", 1): 612.0, ("act", 2): 1038.0,
    ("dve", 1): 658.0, ("dve", 2): 1192.0,
}


def _split_sync_waits(nc, maxw=1):
    """Walrus rejects instructions with >1 sync waits. Hoist extras onto
    single-wait InstNoOp carriers (same engine). The kernel-tail SP drain's
    waits are spread round-robin across all engines (barrier follows)."""
    n_split = n_carriers = 0
    eng_rr = [
        mybir.EngineType.SP,
        mybir.EngineType.Activation,
        mybir.EngineType.DVE,
        mybir.EngineType.PE,
        mybir.EngineType.Pool,
    ]
    for f in nc.m.functions:
        for bb in f.blocks:
            insts = list(bb.instructions)
            out, changed = [], False
            for inst in insts:
                si = inst.sync_info
                waits = list(si.on_wait) if si and si.on_wait else []
                if len(waits) > maxw:
                    n_split += 1
                    changed = True
                    is_tail_drain = (
                        isinstance(inst, mybir.InstDrain)
                        and inst.engine == mybir.EngineType.SP
                        and len(waits) > 2
                    )
                    for k, w in enumerate(waits[:-maxw]):
                        nop = mybir.InstNoOp(name=f"waitnop-{n_carriers}", ins=[], outs=[])
                        n_carriers += 1
                        nop.engine = eng_rr[k % len(eng_rr)] if is_tail_drain else inst.engine
                        nop.sync_info = mybir.SyncInfo(on_wait=[w], on_update=[])
                        out.append(nop)
                    inst.sync_info = mybir.SyncInfo(
                        on_wait=waits[-maxw:], on_update=list(si.on_update or [])
                    )
                out.append(inst)
            if changed:
                bb.instructions = out
    return n_split, n_carriers


def build_nc(n_j=N, n_i=NI):
    f32 = mybir.dt.float32
    fp8 = mybir.dt.float8e4
    u8 = mybir.dt.uint8
    njb = n_j // 128
    nchunks = n_i // CHUNK

    nc = bass.Bass("TRN2", target_bir_lowering=False, debug=False)
    L0 = nc.dram_tensor("L0", [KI, 2, n_j], fp8, kind="ExternalInput").ap()
    L1 = nc.dram_tensor("L1", [KI, 2, n_j], fp8, kind="ExternalInput").ap()
    R0 = nc.dram_tensor("R0", [KI, 2, n_i], fp8, kind="ExternalInput").ap()
    R1 = nc.dram_tensor("R1", [KI, 2, n_i], fp8, kind="ExternalInput").ap()
    # VA[p, 2*jb+m, v] = [V|1] row (jb*128+p), col v — duplicated per pair m.
    VA = nc.dram_tensor("VA", [128, 2 * njb, 129], fp8, kind="ExternalInput").ap()
    VR = nc.dram_tensor("VR", [128, n_i], f32, kind="ExternalInput").ap()
    # Pool-lane operands: j-side scalars (replicated across cores) and i-side
    # replicated rows (per-core) for computing E on GPSIMD entirely in SBUF.
    AS = [nc.dram_tensor(f"AS{m}", [128, 4 * njb], f32, kind="ExternalInput").ap()
          for m in range(2)]
    BR = [[nc.dram_tensor(f"BR{m}{d}", [128, n_i], mybir.dt.float16,
                          kind="ExternalInput").ap() for d in range(3)]
          for m in range(2)]
    CR = [nc.dram_tensor(f"CR{m}", [128, n_i], f32, kind="ExternalInput").ap()
          for m in range(2)]
    OUT = nc.dram_tensor("out", [n_i, DV], f32, kind="ExternalOutput").ap()

    # per-chunk groups: first chunk leads with a single j-block so the first
    # exp starts early; last chunk ends with singles for a short tail drain.
    # POOL_JBS are computed by the GPSIMD lane instead (late in chunk 0 so its
    # operand DMAs have landed).
    POOL_JBS = {0: [], 1: [9, 41], 2: [9, 41], 3: [9, 41]}
    POOL_JBS = {c: POOL_JBS.get(c, []) for c in range(nchunks)}

    def mk_groups(c, first_single, last_singles):
        rem = [j for j in range(njb) if j not in POOL_JBS[c]]
        gs = []
        if first_single:
            gs.append([rem.pop(0)])
            gs.append([rem.pop(0)])
        tail = [rem.pop(), rem.pop()] if last_singles else []
        while len(rem) >= 2:
            gs.append([rem.pop(0), rem.pop(0)])
        if rem:
            gs.append([rem.pop(0)])
        for j in sorted(tail):
            gs.append([j])
        return gs

    with tile.TileContext(nc) as tc:
        with (
            tc.tile_pool(name="resident", bufs=1) as rpool,
            tc.tile_pool(name="eapool", bufs=3, space="PSUM") as eapool,
            tc.tile_pool(name="upool", bufs=2, space="PSUM") as upool,
            tc.tile_pool(name="spool", bufs=6) as spool,
            tc.tile_pool(name="opool", bufs=8) as opool,
            tc.tile_pool(name="scalars", bufs=4) as scpool,
        ):
            # --- resident inputs, spread across DMA-issue rails ---
            l0_t = rpool.tile([KI, 2, n_j], fp8)
            l1_t = rpool.tile([KI, 2, n_j], fp8)
            r0_t = rpool.tile([KI, 2, n_i], fp8)
            r1_t = rpool.tile([KI, 2, n_i], fp8)
            va_t = rpool.tile([128, 2 * njb, 129], fp8)
            vr_t = rpool.tile([128, n_i], f32)
            as_t = [rpool.tile([128, 4 * njb], f32, name=f"as{m}")
                    for m in range(2)]
            br_t = [[rpool.tile([128, n_i], mybir.dt.float16, name=f"br{m}{d}")
                     for d in range(3)] for m in range(2)]
            cr_t = [rpool.tile([128, n_i], f32, name=f"cr{m}") for m in range(2)]

            # Modality-0 inputs ride sync/HWDGE, modality-1 rides the Pool
            # SWDGE path so their configs run in parallel (HWDGE configs
            # serialize at ~625ns on one shared device); ACT issues no DMAs so
            # its sequencer reaches the first exp immediately.
            PIECE = 8  # j-blocks per L piece
            nc.sync.dma_start(r0_t[:], R0)
            nc.sync.dma_start(l0_t[:, :, 0:PIECE * 128], L0[:, :, 0:PIECE * 128])
            nc.gpsimd.dma_start(r1_t[:], R1)
            nc.gpsimd.dma_start(l1_t[:, :, 0:PIECE * 128], L1[:, :, 0:PIECE * 128])
            nc.gpsimd.dma_start(va_t[:, 0:2 * PIECE, :], VA[:, 0:2 * PIECE, :])
            for p in range(PIECE, njb, PIECE):
                nc.sync.dma_start(
                    l0_t[:, :, p * 128:(p + PIECE) * 128],
                    L0[:, :, p * 128:(p + PIECE) * 128],
                )
                nc.sync.dma_start(
                    l1_t[:, :, p * 128:(p + PIECE) * 128],
                    L1[:, :, p * 128:(p + PIECE) * 128],
                )
                nc.gpsimd.dma_start(
                    va_t[:, 2 * p:2 * (p + PIECE), :], VA[:, 2 * p:2 * (p + PIECE), :]
                )
            for m in range(2):
                nc.sync.dma_start(as_t[m][:], AS[m])
                nc.sync.dma_start(cr_t[m][:], CR[m])
                for d in range(3):
                    nc.sync.dma_start(br_t[m][d][:], BR[m][d])
            nc.sync.dma_start(vr_t[:], VR)

            # ACT exp-table preload + PE p-state warm-up during input DMA.
            dummy = scpool.tile([128, 1], f32, tag="dummy")
            nc.vector.memset(dummy[:], 0.0)
            dummy2 = scpool.tile([128, 1], f32, tag="dummy2")
            nc.scalar.activation(dummy2[:], dummy[:], mybir.ActivationFunctionType.Exp)
            dmm = scpool.tile([1, 256], mybir.dt.bfloat16, tag="dmm")
            nc.vector.memset(dmm[:], 0.0)
            e_warm = eapool.tile([128, 4, 256], f32, tag="ea", name="e_warm")
            for _ in range(14):
                nc.tensor.matmul(
                    e_warm[:, 0, :], lhsT=dmm[:, 0:128], rhs=dmm[:, 0:256],
                    start=True, stop=True,
                )

            # --- global stream of (chunk, group) items with greedy ACT/DVE
            # balance; accum matmuls deferred DEFER groups so the PE never
            # stalls on an in-flight exp, including across chunk boundaries.
            chunk_groups = [mk_groups(0, True, False)] + [
                mk_groups(c, False, False) for c in range(1, nchunks - 1)
            ] + [mk_groups(nchunks - 1, False, True)]
            items = [(c, gi) for c in range(nchunks)
                     for gi in range(len(chunk_groups[c]))]
            t_eng = {"act": 0.0, "dve": 0.0}
            u_tiles = {}
            deferred = []
            pool_s = {}   # (c, jb) -> (s0, s1) u8 tiles

            def emit_pool_chain(c, jb):
                c0 = c * CHUNK
                s01 = []
                for m in range(2):
                    e = opool.tile([128, CHUNK], f32, tag="pe",
                                   name=f"pe_{c}_{jb}_{m}")
                    t = opool.tile([128, CHUNK], f32, tag="pt",
                                   name=f"pt_{c}_{jb}_{m}")
                    nc.gpsimd.tensor_scalar(
                        e[:], br_t[m][0][:, c0:c0 + CHUNK],
                        as_t[m][:, 0 * njb + jb:0 * njb + jb + 1],
                        as_t[m][:, 3 * njb + jb:3 * njb + jb + 1],
                        mybir.AluOpType.mult, mybir.AluOpType.add)
                    for d in (1, 2):
                        nc.gpsimd.tensor_scalar(
                            t[:], br_t[m][d][:, c0:c0 + CHUNK],
                            as_t[m][:, d * njb + jb:d * njb + jb + 1], None,
                            mybir.AluOpType.mult)
                        nc.gpsimd.tensor_add(e[:], e[:], t[:])
                    nc.gpsimd.tensor_add(e[:], e[:], cr_t[m][:, c0:c0 + CHUNK])
                    s_m = spool.tile([128, CHUNK], u8, tag=f"ps{m}",
                                     name=f"ps_{c}_{jb}_{m}")
                    nc.gpsimd.tensor_scalar(
                        s_m[:], e[:], SCH_A, SCH_B,
                        mybir.AluOpType.mult, mybir.AluOpType.add)
                    s01.append(s_m)
                pool_s[(c, jb)] = s01

            def emit_pool_accums(c):
                # plain (non-DoubleRow) fp8 matmuls; PE has ample slack
                u_t = u_tiles[c]
                for jb in POOL_JBS[c]:
                    s0, s1 = pool_s.pop((c, jb))
                    for isub in range(2):
                        for m, s_m in ((0, s0), (1, s1)):
                            nc.tensor.matmul(
                                u_t[:, isub * 256:isub * 256 + 129],
                                lhsT=s_m[:, isub * 128:(isub + 1) * 128].bitcast(
                                    mybir.dt.float8e4),
                                rhs=va_t[:, 2 * jb + m:2 * jb + m + 1, :],
                                start=False, stop=False,
                                skip_group_check=True,
                            )

            def emit_accums(item):
                c, s_t, jbs, first, last = item
                if last:
                    emit_pool_accums(c)
                u_t = u_tiles[c]
                for t, jb in enumerate(jbs):
                    for isub in range(2):
                        # DR psum writes need >=1KB-aligned offsets: slots at
                        # f32 cols 0 and 256 of a full bank.
                        nc.tensor.matmul(
                            u_t[:, isub * 256:isub * 256 + 129],
                            lhsT=s_t[:, 2 * t:2 * t + 2,
                                     isub * 128:(isub + 1) * 128].bitcast(
                                         mybir.dt.float8e4),
                            rhs=va_t[:, 2 * jb:2 * jb + 2, :],
                            start=(first and t == 0 and isub == 0),
                            stop=(last and t == len(jbs) - 1 and isub == 1),
                            skip_group_check=True,
                            perf_mode=mybir.MatmulPerfMode.DoubleRow,
                        )
                if last:
                    emit_epilogue(c)

            def emit_epilogue(c):
                # out rows = U/(rowsum+EPS_S) + V residual; residual add and
                # store run on Pool/SP rails, PSUM-side ops on DVE.
                u_t = u_tiles.pop(c)
                for isub in range(2):
                    g = c * 2 + isub
                    ub = u_t[:, isub * 256:isub * 256 + 129]
                    rt = scpool.tile([128, 1], f32, tag="rt", name=f"rt_{g}")
                    nc.vector.tensor_scalar_add(rt[:], ub[:, 128:129], EPS_S)
                    ri = scpool.tile([128, 1], f32, tag="ri", name=f"ri_{g}")
                    nc.vector.reciprocal(ri[:], rt[:])
                    ot = opool.tile([128, DV], f32, tag="ot", name=f"ot_{g}")
                    nc.vector.tensor_scalar_mul(ot[:], ub[:, 0:DV], ri[:])
                    nc.gpsimd.tensor_add(ot[:], ot[:],
                                         vr_t[:, g * 128:(g + 1) * 128])
                    out_eng = (nc.sync if (isub == 0 or c == nchunks - 1)
                               else nc.gpsimd)
                    out_eng.dma_start(OUT[g * 128:(g + 1) * 128, :], ot[:])

            def work(c, gi):
                jbs = chunk_groups[c][gi]
                nt = len(jbs)
                c0 = c * CHUNK
                if gi == 0:
                    u_tiles[c] = upool.tile([128, 512], f32, tag="u",
                                            name=f"u_{c}")
                    if c == 0:
                        for jb in POOL_JBS[0]:
                            emit_pool_chain(0, jb)
                    if c + 1 < nchunks:
                        # next chunk's Pool chains start a whole chunk early
                        # so they always beat their accum due-time
                        for jb in POOL_JBS[c + 1]:
                            emit_pool_chain(c + 1, jb)
                ea = eapool.tile([128, 4, 256], f32, tag="ea",
                                 name=f"ea_{c}_{jbs[0]}")
                for t, jb in enumerate(jbs):
                    for m, (lt, rt) in enumerate(((l0_t, r0_t), (l1_t, r1_t))):
                        nc.tensor.matmul(
                            ea[:, 2 * t + m, :],
                            lhsT=lt[:, :, jb * 128:(jb + 1) * 128],
                            rhs=rt[:, :, c0:c0 + CHUNK],
                            start=True, stop=True,
                            perf_mode=mybir.MatmulPerfMode.DoubleRow,
                        )
                s_t = spool.tile([128, 4, 256], u8, tag="s",
                                 name=f"s_{c}_{jbs[0]}")
                is_final = (c == nchunks - 1 and gi >= len(chunk_groups[c]) - 1)
                eng = ("act" if is_final else
                       min(("act", "dve"), key=lambda e: t_eng[e] + COST[(e, nt)]))
                t_eng[eng] += COST[(eng, nt)]
                if jbs[-1] == njb - 1:
                    t_eng["dve"] += 700.0  # epilogue PSUM-side ops land on DVE
                if eng == "act":
                    nc.scalar.activation(
                        s_t[:, 0:2 * nt, :].bitcast(mybir.dt.float8e4),
                        ea[:, 0:2 * nt, :],
                        mybir.ActivationFunctionType.Exp,
                    )
                else:
                    nc.vector.tensor_scalar(
                        s_t[:, 0:2 * nt, :], ea[:, 0:2 * nt, :], SCH_A, SCH_B,
                        mybir.AluOpType.mult, mybir.AluOpType.add,
                    )
                return (c, s_t, jbs, gi == 0, jbs[-1] == njb - 1)

            for c, gi in items:
                deferred.append(work(c, gi))
                if len(deferred) > DEFER:
                    emit_accums(deferred.pop(0))
            while deferred:
                emit_accums(deferred.pop(0))

    _split_sync_waits(nc)
    return nc


def _split3_e4(v):
    parts = []
    r = np.asarray(v, np.float32)
    for _ in range(3):
        p = r.astype(E4NP).astype(np.float32)
        parts.append(p)
        r = r - p
    return parts


def _build_features(gf, sigma, wv):
    """24 (L_row, R_row) pairs of e4m3 rows s.t. sum_r L_r[j]*R_r[i] =
    -d2[j,i]/(2 sigma^2) + ln(wv) + SCALE_OCT*ln2 to ~1e-2 abs."""
    gf = np.asarray(gf, np.float32)
    n = gf.shape[0]
    g = np.float32(1.0 / (2.0 * sigma * sigma))
    sq = (gf * gf).sum(axis=1)
    a = 2.0 * g * gf
    b = gf
    dh = -g * sq * 0.5
    ch = (-g * sq + np.float32(np.log(wv))
          + np.float32(SCALE_OCT * np.log(2.0))) * 0.5

    A = _split3_e4(a)
    B = _split3_e4(b)
    D = _split3_e4(dh)
    C = _split3_e4(ch)
    ones = np.ones(n, np.float32)
    twos = 2.0 * ones

    Lrows, Rrows = [], []
    for ka, kb in [(0, 0), (0, 1), (1, 0), (1, 1), (0, 2), (2, 0)]:
        for d in range(DG):
            Lrows.append(A[ka][:, d])
            Rrows.append(B[kb][:, d])
    for k in range(3):
        Lrows.append(D[k])
        Rrows.append(twos)
    for k in range(3):
        Lrows.append(twos)
        Rrows.append(C[k])
    assert len(Lrows) == 2 * KI
    L = np.zeros((KI, 2, n), E4NP)
    R = np.zeros((KI, 2, n), E4NP)
    for r in range(2 * KI):
        L[r % KI, r // KI] = Lrows[r].astype(E4NP)
        R[r % KI, r // KI] = Rrows[r].astype(E4NP)
    return L, R


def _pool_operands(gf, sigma, wv, n_cores):
    """GPSIMD-lane operands: AS [128, 4*njb] f32 (a0,a1,a2,d per j), and
    per-core BR (i-side gf rows, f16) / CR (c per i, f32) replicated over
    partitions."""
    gf = np.asarray(gf, np.float32)
    n = gf.shape[0]
    njb = n // 128
    ni = n // n_cores
    g = np.float32(1.0 / (2.0 * sigma * sigma))
    sq = (gf * gf).sum(axis=1)
    a = 2.0 * g * gf                      # [n, 3] j-side
    dterm = -g * sq                       # [n] j-side
    cterm = (-g * sq + np.float32(np.log(wv))
             + np.float32(SCALE_OCT * np.log(2.0)))  # [n] i-side
    AS = np.zeros((128, 4 * njb), np.float32)
    for d in range(3):
        AS[:, d * njb:(d + 1) * njb] = a[:, d].reshape(njb, 128).T
    AS[:, 3 * njb:4 * njb] = dterm.reshape(njb, 128).T
    BRs, CRs = [], []
    for c in range(n_cores):
        rows = slice(c * ni, (c + 1) * ni)
        BRs.append([np.broadcast_to(gf[rows, d].astype(np.float16), (128, ni))
                    .copy() for d in range(3)])
        CRs.append(np.broadcast_to(cterm[rows], (128, ni)).copy())
    return AS, BRs, CRs


def _prepare_inputs(gf0, gf1, node_v_feats, weights, sigmas, n_cores=NCORES):
    weights = np.asarray(weights, np.float32)
    sigmas = np.asarray(sigmas, np.float32)
    V = np.asarray(node_v_feats, np.float32)
    n = V.shape[0]
    ni = n // n_cores
    njb = n // 128

    L0, R0full = _build_features(gf0, float(sigmas[0]), float(weights[0]))
    L1, R1full = _build_features(gf1, float(sigmas[1]), float(weights[1]))
    AS0, BR0s, CR0s = _pool_operands(gf0, float(sigmas[0]), float(weights[0]), n_cores)
    AS1, BR1s, CR1s = _pool_operands(gf1, float(sigmas[1]), float(weights[1]), n_cores)

    vaug = np.concatenate(
        [V.astype(E4NP).astype(np.float32), np.ones((n, 1), np.float32)], axis=1
    ).astype(E4NP)                                    # [n, 129]
    va = np.zeros((128, 2 * njb, 129), E4NP)
    blocks = vaug.reshape(njb, 128, 129)
    for m in range(2):
        va[:, m::2, :] = blocks.transpose(1, 0, 2)

    in_maps = []
    for c in range(n_cores):
        rows = slice(c * ni, (c + 1) * ni)
        vr = np.ascontiguousarray(
            V[rows].reshape(ni // 128, 128, DV).transpose(1, 0, 2).reshape(128, ni)
        )
        in_maps.append({
            "L0": np.ascontiguousarray(L0),
            "L1": np.ascontiguousarray(L1),
            "R0": np.ascontiguousarray(R0full[:, :, rows]),
            "R1": np.ascontiguousarray(R1full[:, :, rows]),
            "VA": va,
            "VR": vr,
            "AS0": AS0, "AS1": AS1,
            "BR00": BR0s[c][0], "BR01": BR0s[c][1], "BR02": BR0s[c][2],
            "BR10": BR1s[c][0], "BR11": BR1s[c][1], "BR12": BR1s[c][2],
            "CR0": CR0s[c], "CR1": CR1s[c],
        })
    return in_maps


_NC_CACHE = {}


def _get_nc(n_j=N, n_i=NI):
    key = (n_j, n_i)
    if key not in _NC_CACHE:
        _NC_CACHE[key] = build_nc(n_j, n_i)
    return _NC_CACHE[key]


_EXEC_CACHE = {}


def _get_executor(nc, n_cores):
    key = (id(nc), n_cores)
    if key in _EXEC_CACHE:
        return _EXEC_CACHE[key]
    import jax
    from jax.experimental.shard_map import shard_map
    from jax.sharding import Mesh, PartitionSpec
    from concourse.bass2jax import (
        install_neuronx_cc_hook,
        _bass_exec_p,
        partition_id_tensor,
    )

    install_neuronx_cc_hook()

    partition_name = nc.partition_id_tensor.name if nc.partition_id_tensor else None
    in_names, out_names, out_avals = [], [], []
    for alloc in nc.m.functions[0].allocations:
        if not isinstance(alloc, mybir.MemoryLocationSet):
            continue
        name = alloc.memorylocations[0].name
        if alloc.kind == "ExternalInput":
            if name != partition_name:
                in_names.append(name)
        elif alloc.kind == "ExternalOutput":
            out_names.append(name)
            out_avals.append(
                jax.core.ShapedArray(tuple(alloc.tensor_shape), mybir.dt.np(alloc.dtype))
            )
    n_params = len(in_names)
    all_names = list(in_names) + list(out_names)
    if partition_name is not None:
        all_names.append(partition_name)

    def _body(*args):
        operands = list(args)
        if partition_name is not None:
            operands.append(partition_id_tensor())
        outs = _bass_exec_p.bind(
            *operands,
            out_avals=tuple(out_avals),
            in_names=tuple(all_names),
            out_names=tuple(out_names),
            lowering_input_output_aliases=(),
            sim_require_finite=True,
            sim_require_nnan=True,
            nc=nc,
        )
        return tuple(outs)

    devices = jax.devices()[:n_cores]
    mesh = Mesh(np.asarray(devices), ("core",))
    n_outs = len(out_names)
    replicated = frozenset(["L0", "L1", "VA", "AS0", "AS1"])
    in_specs = tuple(
        PartitionSpec() if name in replicated else PartitionSpec("core")
        for name in in_names
    ) + (PartitionSpec("core"),) * n_outs
    sharded = jax.jit(
        shard_map(
            _body,
            mesh=mesh,
            in_specs=in_specs,
            out_specs=(PartitionSpec("core"),) * n_outs,
            check_rep=False,
        ),
        donate_argnums=tuple(range(n_params, n_params + n_outs)),
        keep_unused=True,
    )
    entry = (sharded, in_names, out_names, out_avals, replicated)
    _EXEC_CACHE[key] = entry
    return entry


def _run(nc, in_maps, n_cores):
    sharded, in_names, out_names, out_avals, replicated = _get_executor(nc, n_cores)
    concat_in = [
        in_maps[0][name] if name in replicated
        else np.concatenate([in_maps[c][name] for c in range(n_cores)], axis=0)
        for name in in_names
    ]
    concat_zeros = [
        np.zeros((n_cores * a.shape[0], *a.shape[1:]), a.dtype) for a in out_avals
    ]
    out_arrs = sharded(*concat_in, *concat_zeros)
    return [
        {
            name: np.asarray(out_arrs[i]).reshape(n_cores, *out_avals[i].shape)[c]
            for i, name in enumerate(out_names)
        }
        for c in range(n_cores)
    ]


def kernel(gf0, gf1, node_v_feats, weights, sigmas):
    import jax

    in_maps = _prepare_inputs(gf0, gf1, node_v_feats, weights, sigmas)
    nc = _get_nc()
    last_exc = None
    for attempt in range(3):
        try:
            results = _run(nc, in_maps, NCORES)
            jax.effects_barrier()
            out = np.concatenate([results[c]["out"] for c in range(NCORES)], axis=0)
            return np.ascontiguousarray(out.astype(np.float32))
        except Exception as e:
            last_exc = e
            _EXEC_CACHE.clear()
            try:
                jax.clear_caches()
            except Exception:
                pass
            try:
                jax._src.xla_bridge.backends.cache_clear()  # type: ignore[attr-defined]
            except Exception:
                pass
            import time as _time
            _time.sleep(5 * (attempt + 1))
    raise last_exc


# revision 8
# speedup vs baseline: 1.0383x; 1.0009x over previous
"""Trainium2 Bass kernel v2 for nn_GAttn_67147518705771.

Computes: score = w0*RBF(gf0, s0) + w1*RBF(gf1, s1)  (N x N)
          attn  = score / (rowsum(score) + 0.01)
          out   = attn @ V + V

Row-parallel over 8 cores; core c owns output rows [c*1024, (c+1)*1024).

v2 strategy (vs the f16 baseline):
  * All matmuls are fp8e4 + DoubleRow (0.5 cyc/col, 256-row contraction):
    - E matmul per modality: 24 e4m3 feature rows as [12, 2, *] pairs
      produce E[j,i]*1 with ln(w) + 5*ln2 folded in (S scaled x32 so the
      e4m3 subnormal band sits harmlessly low; epilogue uses EPS*32).
    - Accumulation pairs the TWO MODALITY S tiles in one DoubleRow matmul
      against a byte-duplicated VA block [128, 2, 129] ([V|1] e4m3), so
      S0@VA + S1@VA costs 64.5 PE cycles per (jb, i128) block.
  * exp runs on ACT (native Exp -> fp8 out) and DVE (Schraudolph affine ->
    u8, bitcast e4m3). GPSIMD has no PSUM port so it only issues DMAs.
  * PSUM: 3 x [128,4,256] E tiles (2 banks each) + 2 x U accumulator banks.
"""

import numpy as np
import ml_dtypes

import concourse.bass as bass
import concourse.tile as tile
import concourse.mybir as mybir

E4NP = ml_dtypes.float8_e4m3
EPS = 0.01
N = 8192
DG = 3
DV = 128
NCORES = 8
NI = N // NCORES
KI = 12            # feature pair-rows per modality (24 rows via DoubleRow)
CHUNK = 256
NJB = N // 128
SCALE_OCT = 5      # S scaled by 2^5; epilogue divides by (rowsum + EPS*32)
EPS_S = EPS * (1 << SCALE_OCT)
SCH_A = float(np.float32(8.0 * 1.4426950408889634))
SCH_B = float(np.float32(55.537))
DEFER = 3          # groups between exp emission and its accum matmuls

# per-group exp cost estimates (ns) for greedy ACT/DVE balancing
COST = {
    ("act", 1): 612.0, ("act", 2): 1038.0,
    ("dve", 1): 658.0, ("dve", 2): 1192.0,
}


def _split_sync_waits(nc, maxw=1):
    """Walrus rejects instructions with >1 sync waits. Hoist extras onto
    single-wait InstNoOp carriers (same engine). The kernel-tail SP drain's
    waits are spread round-robin across all engines (barrier follows)."""
    n_split = n_carriers = 0
    eng_rr = [
        mybir.EngineType.SP,
        mybir.EngineType.Activation,
        mybir.EngineType.DVE,
        mybir.EngineType.PE,
        mybir.EngineType.Pool,
    ]
    for f in nc.m.functions:
        for bb in f.blocks:
            insts = list(bb.instructions)
            out, changed = [], False
            for inst in insts:
                si = inst.sync_info
                waits = list(si.on_wait) if si and si.on_wait else []
                if len(waits) > maxw:
                    n_split += 1
                    changed = True
                    is_tail_drain = (
                        isinstance(inst, mybir.InstDrain)
                        and inst.engine == mybir.EngineType.SP
                        and len(waits) > 2
                    )
                    for k, w in enumerate(waits[:-maxw]):
                        nop = mybir.InstNoOp(name=f"waitnop-{n_carriers}", ins=[], outs=[])
                        n_carriers += 1
                        nop.engine = eng_rr[k % len(eng_rr)] if is_tail_drain else inst.engine
                        nop.sync_info = mybir.SyncInfo(on_wait=[w], on_update=[])
                        out.append(nop)
                    inst.sync_info = mybir.SyncInfo(
                        on_wait=waits[-maxw:], on_update=list(si.on_update or [])
                    )
                out.append(inst)
            if changed:
                bb.instructions = out
    return n_split, n_carriers


def build_nc(n_j=N, n_i=NI):
    f32 = mybir.dt.float32
    fp8 = mybir.dt.float8e4
    u8 = mybir.dt.uint8
    njb = n_j // 128
    nchunks = n_i // CHUNK

    nc = bass.Bass("TRN2", target_bir_lowering=False, debug=False)
    L0 = nc.dram_tensor("L0", [KI, 2, n_j], fp8, kind="ExternalInput").ap()
    L1 = nc.dram_tensor("L1", [KI, 2, n_j], fp8, kind="ExternalInput").ap()
    R0 = nc.dram_tensor("R0", [KI, 2, n_i], fp8, kind="ExternalInput").ap()
    R1 = nc.dram_tensor("R1", [KI, 2, n_i], fp8, kind="ExternalInput").ap()
    # VA[p, 2*jb+m, v] = [V|1] row (jb*128+p), col v — duplicated per pair m.
    VA = nc.dram_tensor("VA", [128, 2 * njb, 129], fp8, kind="ExternalInput").ap()
    VR = nc.dram_tensor("VR", [128, n_i], f32, kind="ExternalInput").ap()
    # Pool-lane operands: j-side scalars (replicated across cores) and i-side
    # replicated rows (per-core) for computing E on GPSIMD entirely in SBUF.
    AS = [nc.dram_tensor(f"AS{m}", [128, 4 * njb], f32, kind="ExternalInput").ap()
          for m in range(2)]
    BR = [[nc.dram_tensor(f"BR{m}{d}", [128, n_i], mybir.dt.float16,
                          kind="ExternalInput").ap() for d in range(3)]
          for m in range(2)]
    CR = [nc.dram_tensor(f"CR{m}", [128, n_i], f32, kind="ExternalInput").ap()
          for m in range(2)]
    OUT = nc.dram_tensor("out", [n_i, DV], f32, kind="ExternalOutput").ap()

    # per-chunk groups: first chunk leads with a single j-block so the first
    # exp starts early; last chunk ends with singles for a short tail drain.
    # POOL_JBS are computed by the GPSIMD lane instead (late in chunk 0 so its
    # operand DMAs have landed).
    POOL_JBS = {0: [], 1: [9, 41], 2: [9, 41], 3: [9, 41]}
    POOL_JBS = {c: POOL_JBS.get(c, []) for c in range(nchunks)}

    def mk_groups(c, first_single, last_singles):
        rem = [j for j in range(njb) if j not in POOL_JBS[c]]
        gs = []
        if first_single:
            gs.append([rem.pop(0)])
            gs.append([rem.pop(0)])
        tail = [rem.pop(), rem.pop()] if last_singles else []
        while len(rem) >= 2:
            gs.append([rem.pop(0), rem.pop(0)])
        if rem:
            gs.append([rem.pop(0)])
        for j in sorted(tail):
            gs.append([j])
        return gs

    with tile.TileContext(nc) as tc:
        with (
            tc.tile_pool(name="resident", bufs=1) as rpool,
            tc.tile_pool(name="eapool", bufs=3, space="PSUM") as eapool,
            tc.tile_pool(name="upool", bufs=2, space="PSUM") as upool,
            tc.tile_pool(name="spool", bufs=6) as spool,
            tc.tile_pool(name="opool", bufs=8) as opool,
            tc.tile_pool(name="scalars", bufs=4) as scpool,
        ):
            # --- resident inputs, spread across DMA-issue rails ---
            l0_t = rpool.tile([KI, 2, n_j], fp8)
            l1_t = rpool.tile([KI, 2, n_j], fp8)
            r0_t = rpool.tile([KI, 2, n_i], fp8)
            r1_t = rpool.tile([KI, 2, n_i], fp8)
            va_t = rpool.tile([128, 2 * njb, 129], fp8)
            vr_t = rpool.tile([128, n_i], f32)
            as_t = [rpool.tile([128, 4 * njb], f32, name=f"as{m}")
                    for m in range(2)]
            br_t = [[rpool.tile([128, n_i], mybir.dt.float16, name=f"br{m}{d}")
                     for d in range(3)] for m in range(2)]
            cr_t = [rpool.tile([128, n_i], f32, name=f"cr{m}") for m in range(2)]

            # Modality-0 inputs ride sync/HWDGE, modality-1 rides the Pool
            # SWDGE path so their configs run in parallel (HWDGE configs
            # serialize at ~625ns on one shared device); ACT issues no DMAs so
            # its sequencer reaches the first exp immediately.
            PIECE = 8  # j-blocks per L piece
            nc.sync.dma_start(r0_t[:], R0)
            nc.sync.dma_start(l0_t[:, :, 0:PIECE * 128], L0[:, :, 0:PIECE * 128])
            nc.gpsimd.dma_start(r1_t[:], R1)
            nc.gpsimd.dma_start(l1_t[:, :, 0:PIECE * 128], L1[:, :, 0:PIECE * 128])
            nc.gpsimd.dma_start(va_t[:, 0:2 * PIECE, :], VA[:, 0:2 * PIECE, :])
            for p in range(PIECE, njb, PIECE):
                nc.sync.dma_start(
                    l0_t[:, :, p * 128:(p + PIECE) * 128],
                    L0[:, :, p * 128:(p + PIECE) * 128],
                )
                nc.sync.dma_start(
                    l1_t[:, :, p * 128:(p + PIECE) * 128],
                    L1[:, :, p * 128:(p + PIECE) * 128],
                )
                nc.gpsimd.dma_start(
                    va_t[:, 2 * p:2 * (p + PIECE), :], VA[:, 2 * p:2 * (p + PIECE), :]
                )
            for m in range(2):
                nc.sync.dma_start(as_t[m][:], AS[m])
                nc.sync.dma_start(cr_t[m][:], CR[m])
                for d in range(3):
                    nc.sync.dma_start(br_t[m][d][:], BR[m][d])
            nc.sync.dma_start(vr_t[:], VR)

            # ACT exp-table preload + PE p-state warm-up during input DMA.
            dummy = scpool.tile([128, 1], f32, tag="dummy")
            nc.vector.memset(dummy[:], 0.0)
            dummy2 = scpool.tile([128, 1], f32, tag="dummy2")
            nc.scalar.activation(dummy2[:], dummy[:], mybir.ActivationFunctionType.Exp)
            dmm = scpool.tile([1, 256], mybir.dt.bfloat16, tag="dmm")
            nc.vector.memset(dmm[:], 0.0)
            e_warm = eapool.tile([128, 4, 256], f32, tag="ea", name="e_warm")
            for _ in range(12):
                nc.tensor.matmul(
                    e_warm[:, 0, :], lhsT=dmm[:, 0:128], rhs=dmm[:, 0:256],
                    start=True, stop=True,
                )

            # --- global stream of (chunk, group) items with greedy ACT/DVE
            # balance; accum matmuls deferred DEFER groups so the PE never
            # stalls on an in-flight exp, including across chunk boundaries.
            chunk_groups = [mk_groups(0, True, False)] + [
                mk_groups(c, False, False) for c in range(1, nchunks - 1)
            ] + [mk_groups(nchunks - 1, False, True)]
            items = [(c, gi) for c in range(nchunks)
                     for gi in range(len(chunk_groups[c]))]
            t_eng = {"act": 0.0, "dve": 0.0}
            u_tiles = {}
            deferred = []
            pool_s = {}   # (c, jb) -> (s0, s1) u8 tiles

            def emit_pool_chain(c, jb):
                c0 = c * CHUNK
                s01 = []
                for m in range(2):
                    e = opool.tile([128, CHUNK], f32, tag="pe",
                                   name=f"pe_{c}_{jb}_{m}")
                    t = opool.tile([128, CHUNK], f32, tag="pt",
                                   name=f"pt_{c}_{jb}_{m}")
                    nc.gpsimd.tensor_scalar(
                        e[:], br_t[m][0][:, c0:c0 + CHUNK],
                        as_t[m][:, 0 * njb + jb:0 * njb + jb + 1],
                        as_t[m][:, 3 * njb + jb:3 * njb + jb + 1],
                        mybir.AluOpType.mult, mybir.AluOpType.add)
                    for d in (1, 2):
                        nc.gpsimd.tensor_scalar(
                            t[:], br_t[m][d][:, c0:c0 + CHUNK],
                            as_t[m][:, d * njb + jb:d * njb + jb + 1], None,
                            mybir.AluOpType.mult)
                        nc.gpsimd.tensor_add(e[:], e[:], t[:])
                    nc.gpsimd.tensor_add(e[:], e[:], cr_t[m][:, c0:c0 + CHUNK])
                    s_m = spool.tile([128, CHUNK], u8, tag=f"ps{m}",
                                     name=f"ps_{c}_{jb}_{m}")
                    nc.gpsimd.tensor_scalar(
                        s_m[:], e[:], SCH_A, SCH_B,
                        mybir.AluOpType.mult, mybir.AluOpType.add)
                    s01.append(s_m)
                pool_s[(c, jb)] = s01

            def emit_pool_accums(c):
                # plain (non-DoubleRow) fp8 matmuls; PE has ample slack
                u_t = u_tiles[c]
                for jb in POOL_JBS[c]:
                    s0, s1 = pool_s.pop((c, jb))
                    for isub in range(2):
                        for m, s_m in ((0, s0), (1, s1)):
                            nc.tensor.matmul(
                                u_t[:, isub * 256:isub * 256 + 129],
                                lhsT=s_m[:, isub * 128:(isub + 1) * 128].bitcast(
                                    mybir.dt.float8e4),
                                rhs=va_t[:, 2 * jb + m:2 * jb + m + 1, :],
                                start=False, stop=False,
                                skip_group_check=True,
                            )

            def emit_accums(item):
                c, s_t, jbs, first, last = item
                if last:
                    emit_pool_accums(c)
                u_t = u_tiles[c]
                for t, jb in enumerate(jbs):
                    for isub in range(2):
                        # DR psum writes need >=1KB-aligned offsets: slots at
                        # f32 cols 0 and 256 of a full bank.
                        nc.tensor.matmul(
                            u_t[:, isub * 256:isub * 256 + 129],
                            lhsT=s_t[:, 2 * t:2 * t + 2,
                                     isub * 128:(isub + 1) * 128].bitcast(
                                         mybir.dt.float8e4),
                            rhs=va_t[:, 2 * jb:2 * jb + 2, :],
                            start=(first and t == 0 and isub == 0),
                            stop=(last and t == len(jbs) - 1 and isub == 1),
                            skip_group_check=True,
                            perf_mode=mybir.MatmulPerfMode.DoubleRow,
                        )
                if last:
                    emit_epilogue(c)

            def emit_epilogue(c):
                # out rows = U/(rowsum+EPS_S) + V residual; residual add and
                # store run on Pool/SP rails, PSUM-side ops on DVE.
                u_t = u_tiles.pop(c)
                for isub in range(2):
                    g = c * 2 + isub
                    ub = u_t[:, isub * 256:isub * 256 + 129]
                    rt = scpool.tile([128, 1], f32, tag="rt", name=f"rt_{g}")
                    nc.vector.tensor_scalar_add(rt[:], ub[:, 128:129], EPS_S)
                    ri = scpool.tile([128, 1], f32, tag="ri", name=f"ri_{g}")
                    nc.vector.reciprocal(ri[:], rt[:])
                    ot = opool.tile([128, DV], f32, tag="ot", name=f"ot_{g}")
                    nc.vector.tensor_scalar_mul(ot[:], ub[:, 0:DV], ri[:])
                    nc.gpsimd.tensor_add(ot[:], ot[:],
                                         vr_t[:, g * 128:(g + 1) * 128])
                    out_eng = (nc.sync if (isub == 0 or c == nchunks - 1)
                               else nc.gpsimd)
                    out_eng.dma_start(OUT[g * 128:(g + 1) * 128, :], ot[:])

            def work(c, gi):
                jbs = chunk_groups[c][gi]
                nt = len(jbs)
                c0 = c * CHUNK
                if gi == 0:
                    u_tiles[c] = upool.tile([128, 512], f32, tag="u",
                                            name=f"u_{c}")
                    if c == 0:
                        for jb in POOL_JBS[0]:
                            emit_pool_chain(0, jb)
                    if c + 1 < nchunks:
                        # next chunk's Pool chains start a whole chunk early
                        # so they always beat their accum due-time
                        for jb in POOL_JBS[c + 1]:
                            emit_pool_chain(c + 1, jb)
                ea = eapool.tile([128, 4, 256], f32, tag="ea",
                                 name=f"ea_{c}_{jbs[0]}")
                for t, jb in enumerate(jbs):
                    for m, (lt, rt) in enumerate(((l0_t, r0_t), (l1_t, r1_t))):
                        nc.tensor.matmul(
                            ea[:, 2 * t + m, :],
                            lhsT=lt[:, :, jb * 128:(jb + 1) * 128],
                            rhs=rt[:, :, c0:c0 + CHUNK],
                            start=True, stop=True,
                            perf_mode=mybir.MatmulPerfMode.DoubleRow,
                        )
                s_t = spool.tile([128, 4, 256], u8, tag="s",
                                 name=f"s_{c}_{jbs[0]}")
                is_final = (c == nchunks - 1 and gi >= len(chunk_groups[c]) - 1)
                eng = ("act" if is_final else
                       min(("act", "dve"), key=lambda e: t_eng[e] + COST[(e, nt)]))
                t_eng[eng] += COST[(eng, nt)]
                if jbs[-1] == njb - 1:
                    t_eng["dve"] += 700.0  # epilogue PSUM-side ops land on DVE
                if eng == "act":
                    nc.scalar.activation(
                        s_t[:, 0:2 * nt, :].bitcast(mybir.dt.float8e4),
                        ea[:, 0:2 * nt, :],
                        mybir.ActivationFunctionType.Exp,
                    )
                else:
                    nc.vector.tensor_scalar(
                        s_t[:, 0:2 * nt, :], ea[:, 0:2 * nt, :], SCH_A, SCH_B,
                        mybir.AluOpType.mult, mybir.AluOpType.add,
                    )
                return (c, s_t, jbs, gi == 0, jbs[-1] == njb - 1)

            for c, gi in items:
                deferred.append(work(c, gi))
                if len(deferred) > DEFER:
                    emit_accums(deferred.pop(0))
            while deferred:
                emit_accums(deferred.pop(0))

    _split_sync_waits(nc)
    return nc


def _split3_e4(v):
    parts = []
    r = np.asarray(v, np.float32)
    for _ in range(3):
        p = r.astype(E4NP).astype(np.float32)
        parts.append(p)
        r = r - p
    return parts


def _build_features(gf, sigma, wv):
    """24 (L_row, R_row) pairs of e4m3 rows s.t. sum_r L_r[j]*R_r[i] =
    -d2[j,i]/(2 sigma^2) + ln(wv) + SCALE_OCT*ln2 to ~1e-2 abs."""
    gf = np.asarray(gf, np.float32)
    n = gf.shape[0]
    g = np.float32(1.0 / (2.0 * sigma * sigma))
    sq = (gf * gf).sum(axis=1)
    a = 2.0 * g * gf
    b = gf
    dh = -g * sq * 0.5
    ch = (-g * sq + np.float32(np.log(wv))
          + np.float32(SCALE_OCT * np.log(2.0))) * 0.5

    A = _split3_e4(a)
    B = _split3_e4(b)
    D = _split3_e4(dh)
    C = _split3_e4(ch)
    ones = np.ones(n, np.float32)
    twos = 2.0 * ones

    Lrows, Rrows = [], []
    for ka, kb in [(0, 0), (0, 1), (1, 0), (1, 1), (0, 2), (2, 0)]:
        for d in range(DG):
            Lrows.append(A[ka][:, d])
            Rrows.append(B[kb][:, d])
    for k in range(3):
        Lrows.append(D[k])
        Rrows.append(twos)
    for k in range(3):
        Lrows.append(twos)
        Rrows.append(C[k])
    assert len(Lrows) == 2 * KI
    L = np.zeros((KI, 2, n), E4NP)
    R = np.zeros((KI, 2, n), E4NP)
    for r in range(2 * KI):
        L[r % KI, r // KI] = Lrows[r].astype(E4NP)
        R[r % KI, r // KI] = Rrows[r].astype(E4NP)
    return L, R


def _pool_operands(gf, sigma, wv, n_cores):
    """GPSIMD-lane operands: AS [128, 4*njb] f32 (a0,a1,a2,d per j), and
    per-core BR (i-side gf rows, f16) / CR (c per i, f32) replicated over
    partitions."""
    gf = np.asarray(gf, np.float32)
    n = gf.shape[0]
    njb = n // 128
    ni = n // n_cores
    g = np.float32(1.0 / (2.0 * sigma * sigma))
    sq = (gf * gf).sum(axis=1)
    a = 2.0 * g * gf                      # [n, 3] j-side
    dterm = -g * sq                       # [n] j-side
    cterm = (-g * sq + np.float32(np.log(wv))
             + np.float32(SCALE_OCT * np.log(2.0)))  # [n] i-side
    AS = np.zeros((128, 4 * njb), np.float32)
    for d in range(3):
        AS[:, d * njb:(d + 1) * njb] = a[:, d].reshape(njb, 128).T
    AS[:, 3 * njb:4 * njb] = dterm.reshape(njb, 128).T
    BRs, CRs = [], []
    for c in range(n_cores):
        rows = slice(c * ni, (c + 1) * ni)
        BRs.append([np.broadcast_to(gf[rows, d].astype(np.float16), (128, ni))
                    .copy() for d in range(3)])
        CRs.append(np.broadcast_to(cterm[rows], (128, ni)).copy())
    return AS, BRs, CRs


def _prepare_inputs(gf0, gf1, node_v_feats, weights, sigmas, n_cores=NCORES):
    weights = np.asarray(weights, np.float32)
    sigmas = np.asarray(sigmas, np.float32)
    V = np.asarray(node_v_feats, np.float32)
    n = V.shape[0]
    ni = n // n_cores
    njb = n // 128

    L0, R0full = _build_features(gf0, float(sigmas[0]), float(weights[0]))
    L1, R1full = _build_features(gf1, float(sigmas[1]), float(weights[1]))
    AS0, BR0s, CR0s = _pool_operands(gf0, float(sigmas[0]), float(weights[0]), n_cores)
    AS1, BR1s, CR1s = _pool_operands(gf1, float(sigmas[1]), float(weights[1]), n_cores)

    vaug = np.concatenate(
        [V.astype(E4NP).astype(np.float32), np.ones((n, 1), np.float32)], axis=1
    ).astype(E4NP)                                    # [n, 129]
    va = np.zeros((128, 2 * njb, 129), E4NP)
    blocks = vaug.reshape(njb, 128, 129)
    for m in range(2):
        va[:, m::2, :] = blocks.transpose(1, 0, 2)

    in_maps = []
    for c in range(n_cores):
        rows = slice(c * ni, (c + 1) * ni)
        vr = np.ascontiguousarray(
            V[rows].reshape(ni // 128, 128, DV).transpose(1, 0, 2).reshape(128, ni)
        )
        in_maps.append({
            "L0": np.ascontiguousarray(L0),
            "L1": np.ascontiguousarray(L1),
            "R0": np.ascontiguousarray(R0full[:, :, rows]),
            "R1": np.ascontiguousarray(R1full[:, :, rows]),
            "VA": va,
            "VR": vr,
            "AS0": AS0, "AS1": AS1,
            "BR00": BR0s[c][0], "BR01": BR0s[c][1], "BR02": BR0s[c][2],
            "BR10": BR1s[c][0], "BR11": BR1s[c][1], "BR12": BR1s[c][2],
            "CR0": CR0s[c], "CR1": CR1s[c],
        })
    return in_maps


_NC_CACHE = {}


def _get_nc(n_j=N, n_i=NI):
    key = (n_j, n_i)
    if key not in _NC_CACHE:
        _NC_CACHE[key] = build_nc(n_j, n_i)
    return _NC_CACHE[key]


_EXEC_CACHE = {}


def _get_executor(nc, n_cores):
    key = (id(nc), n_cores)
    if key in _EXEC_CACHE:
        return _EXEC_CACHE[key]
    import jax
    from jax.experimental.shard_map import shard_map
    from jax.sharding import Mesh, PartitionSpec
    from concourse.bass2jax import (
        install_neuronx_cc_hook,
        _bass_exec_p,
        partition_id_tensor,
    )

    install_neuronx_cc_hook()

    partition_name = nc.partition_id_tensor.name if nc.partition_id_tensor else None
    in_names, out_names, out_avals = [], [], []
    for alloc in nc.m.functions[0].allocations:
        if not isinstance(alloc, mybir.MemoryLocationSet):
            continue
        name = alloc.memorylocations[0].name
        if alloc.kind == "ExternalInput":
            if name != partition_name:
                in_names.append(name)
        elif alloc.kind == "ExternalOutput":
            out_names.append(name)
            out_avals.append(
                jax.core.ShapedArray(tuple(alloc.tensor_shape), mybir.dt.np(alloc.dtype))
            )
    n_params = len(in_names)
    all_names = list(in_names) + list(out_names)
    if partition_name is not None:
        all_names.append(partition_name)

    def _body(*args):
        operands = list(args)
        if partition_name is not None:
            operands.append(partition_id_tensor())
        outs = _bass_exec_p.bind(
            *operands,
            out_avals=tuple(out_avals),
            in_names=tuple(all_names),
            out_names=tuple(out_names),
            lowering_input_output_aliases=(),
            sim_require_finite=True,
            sim_require_nnan=True,
            nc=nc,
        )
        return tuple(outs)

    devices = jax.devices()[:n_cores]
    mesh = Mesh(np.asarray(devices), ("core",))
    n_outs = len(out_names)
    replicated = frozenset(["L0", "L1", "VA", "AS0", "AS1"])
    in_specs = tuple(
        PartitionSpec() if name in replicated else PartitionSpec("core")
        for name in in_names
    ) + (PartitionSpec("core"),) * n_outs
    sharded = jax.jit(
        shard_map(
            _body,
            mesh=mesh,
            in_specs=in_specs,
            out_specs=(PartitionSpec("core"),) * n_outs,
            check_rep=False,
        ),
        donate_argnums=tuple(range(n_params, n_params + n_outs)),
        keep_unused=True,
    )
    entry = (sharded, in_names, out_names, out_avals, replicated)
    _EXEC_CACHE[key] = entry
    return entry


def _run(nc, in_maps, n_cores):
    sharded, in_names, out_names, out_avals, replicated = _get_executor(nc, n_cores)
    concat_in = [
        in_maps[0][name] if name in replicated
        else np.concatenate([in_maps[c][name] for c in range(n_cores)], axis=0)
        for name in in_names
    ]
    concat_zeros = [
        np.zeros((n_cores * a.shape[0], *a.shape[1:]), a.dtype) for a in out_avals
    ]
    out_arrs = sharded(*concat_in, *concat_zeros)
    return [
        {
            name: np.asarray(out_arrs[i]).reshape(n_cores, *out_avals[i].shape)[c]
            for i, name in enumerate(out_names)
        }
        for c in range(n_cores)
    ]


def kernel(gf0, gf1, node_v_feats, weights, sigmas):
    import jax

    in_maps = _prepare_inputs(gf0, gf1, node_v_feats, weights, sigmas)
    nc = _get_nc()
    last_exc = None
    for attempt in range(3):
        try:
            results = _run(nc, in_maps, NCORES)
            jax.effects_barrier()
            out = np.concatenate([results[c]["out"] for c in range(NCORES)], axis=0)
            return np.ascontiguousarray(out.astype(np.float32))
        except Exception as e:
            last_exc = e
            _EXEC_CACHE.clear()
            try:
                jax.clear_caches()
            except Exception:
                pass
            try:
                jax._src.xla_bridge.backends.cache_clear()  # type: ignore[attr-defined]
            except Exception:
                pass
            import time as _time
            _time.sleep(5 * (attempt + 1))
    raise last_exc
